# revision 7
# baseline (speedup 1.0000x reference)
"""Trainium2 Bass kernel for nn_Block_66073776882206 (ragged_sequence).

Strategy
--------
Pure data parallelism over the batch: pad 500 -> 512, shard 64 samples per
NeuronCore across 8 cores, replicate all weights. Everything on-device is
feature-major: SBUF tiles are (features<=128, batch-cols), LSTM cell matmuls
use lhsT = weight chunks (K=feat_in, M=gate_rows), rhs = activations
(K, N=batch-cols), PSUM out (gate_rows, batch-cols).

The heavy part (shift_accumulate) runs, for each backcast step f in [0,24),
a two-segment bidirectional 2-layer LSTM over the length-24 sequence. The
f-axis is batched into the matmul N dimension: N = 24 f-blocks x 64 batch =
1536 columns. The permutation perm_f[t] = (f-1-t if t<f else 23-(t-f)) is
linear in f with slope 1 in both branches, so every permuted read/write
reduces to a contiguous slice or a stride-1600 access pattern on a
step-major store -- no gather DMAs. The backward L1 run for block f only
needs steps 0..f (its consumed output is the processing-step-f entry), so
it runs on a shrinking column suffix (~52% of the full work).

Gate order is host-reordered to [i, f, o, g] so one ACT instruction applies
sigmoid across i,f,o and one applies tanh to g. Biases are folded into the
x-side matmul via an appended ones-row on the rhs / bias-row on the lhsT.

Matmul inputs are bf16; PSUM accumulation and the cell state c stay fp32.
"""
import sys
import os

sys.path.insert(0, "/opt/trn_rl_repo")

import numpy as np
import ml_dtypes

import concourse.bass as bass
import concourse.bacc as bacc
import concourse.mybir as mybir
from concourse.tile import TileContext
from concourse.bass_utils import run_bass_kernel_spmd

F32 = mybir.dt.float32
BF16 = mybir.dt.bfloat16
AF = mybir.ActivationFunctionType
ALU = mybir.AluOpType

NV = 8
H = 100          # UNITS
T = 24           # BACKLEN
FC = 12          # FORECAST
B = 500
NCORES = 8
BC = 64          # batch per core (padded)
BP = NCORES * BC  # 512
NF = T * BC      # 1536 f-batched columns
STRIDE = NF + BC  # 1600: f-block stride between (step, block) diagonals
H2 = 2 * H

WDT = BF16
NP_WDT = ml_dtypes.bfloat16

GATE_PERM = [0, 1, 3, 2]  # reference order [i, f, g, o] -> ours [i, f, o, g]

ENC_WNAMES = ["wi0", "wh0", "wi1a", "wi1b", "wh1"]
D_WNAMES = ["wi0a", "wi0b", "wh0", "wi1a", "wi1b", "wh1"]
DS_WNAMES = ["wi0a", "wi0b", "wi0c", "wi0d", "wh0a", "wh0b",
             "wi1a", "wi1b", "wi1c", "wi1d", "wh1a", "wh1b"]


# ----------------------------------------------------------------------------
# host-side weight prep
# ----------------------------------------------------------------------------

def _reorder(M):
    """(4H, X) -> rows gate-reordered to [i, f, o, g]."""
    Hq = M.shape[0] // 4
    return M.reshape(4, Hq, -1)[GATE_PERM].reshape(4 * Hq, -1)


def _wi_aug(Wi, b, zero_ch=()):
    """W_ih (4H, I), b (4H,) -> lhsT (I+1, 4H): rows = input features + a
    bias row (applied via the ones-row in rhs); cols = reordered gate rows."""
    W = np.array(Wi, dtype=np.float32).copy()
    for c in zero_ch:
        W[:, c] = 0.0
    W = _reorder(W)
    bb = _reorder(np.asarray(b, np.float32).reshape(-1, 1))
    return np.concatenate([W.T, bb.T], axis=0)


def _wh(Wh):
    return np.ascontiguousarray(_reorder(np.asarray(Wh, np.float32)).T)


def _enc_pack(p, zero_ch):
    """2-layer bidir LSTM with H=100, I=8 -> dict of (K, 2*400) arrays."""
    out = {k: [] for k in ENC_WNAMES}
    for d in range(2):
        full0 = _wi_aug(p["W_ih0"][d], p["b0"][d], zero_ch)      # (9, 400)
        out["wi0"].append(full0)
        out["wh0"].append(_wh(p["W_hh0"][d]))                     # (100, 400)
        full1 = _wi_aug(p["W_ih1"][d], p["b1"][d])                # (201, 400)
        out["wi1a"].append(np.concatenate([full1[0:H], full1[2 * H:2 * H + 1]], 0))
        out["wi1b"].append(full1[H:2 * H])
        out["wh1"].append(_wh(p["W_hh1"][d]))
    return {k: np.concatenate(v, 1) for k, v in out.items()}


def _dec_pack(p):
    """p_dec: H=100, I=200."""
    out = {k: [] for k in D_WNAMES}
    for d in range(2):
        full0 = _wi_aug(p["W_ih0"][d], p["b0"][d])                # (201, 400)
        out["wi0a"].append(np.concatenate([full0[0:H], full0[2 * H:2 * H + 1]], 0))
        out["wi0b"].append(full0[H:2 * H])
        out["wh0"].append(_wh(p["W_hh0"][d]))
        full1 = _wi_aug(p["W_ih1"][d], p["b1"][d])
        out["wi1a"].append(np.concatenate([full1[0:H], full1[2 * H:2 * H + 1]], 0))
        out["wi1b"].append(full1[H:2 * H])
        out["wh1"].append(_wh(p["W_hh1"][d]))
    return {k: np.concatenate(v, 1) for k, v in out.items()}


def _decS_pack(p):
    """p_decS / p_decSC: H=200, I=400, 4H=800. K-chunks of 100 rows."""
    out = {k: [] for k in DS_WNAMES}
    for d in range(2):
        full0 = _wi_aug(p["W_ih0"][d], p["b0"][d])                # (401, 800)
        out["wi0a"].append(np.concatenate([full0[0:100], full0[400:401]], 0))
        out["wi0b"].append(full0[100:200])
        out["wi0c"].append(full0[200:300])
        out["wi0d"].append(full0[300:400])
        w0 = _wh(p["W_hh0"][d])                                    # (200, 800)
        out["wh0a"].append(w0[0:100])
        out["wh0b"].append(w0[100:200])
        full1 = _wi_aug(p["W_ih1"][d], p["b1"][d])
        out["wi1a"].append(np.concatenate([full1[0:100], full1[400:401]], 0))
        out["wi1b"].append(full1[100:200])
        out["wi1c"].append(full1[200:300])
        out["wi1d"].append(full1[300:400])
        w1 = _wh(p["W_hh1"][d])
        out["wh1a"].append(w1[0:100])
        out["wh1b"].append(w1[100:200])
    return {k: np.concatenate(v, 1) for k, v in out.items()}  # (100/101, 1600)


def _prep_inputs(xt, xorig, p_lstm, p_dec, p_lstmS, p_decS, p_lstmSC, p_decSC,
                 lin_w, lin_b, linS_w, linS_b):
    """Build the per-core input maps. Weights replicated; x sharded."""
    xt = np.asarray(xt, np.float32)
    xorig = np.asarray(xorig, np.float32)
    xt_p = np.zeros((BP, T, NV), np.float32)
    xo_p = np.zeros((BP, T, NV), np.float32)
    xt_p[:B] = xt
    xo_p[:B] = xorig

    weights = {}
    for pref, pk, zc in [("e", p_lstm, (1, 2)), ("s", p_lstmS, (2,)),
                         ("c", p_lstmSC, (1,))]:
        for k, v in _enc_pack(pk, zc).items():
            weights[pref + "_" + k] = v.astype(NP_WDT)
    for k, v in _dec_pack(p_dec).items():
        weights["d_" + k] = v.astype(NP_WDT)
    for k, v in _decS_pack(p_decS).items():
        weights["ds_" + k] = v.astype(NP_WDT)
    for k, v in _decS_pack(p_decSC).items():
        weights["dc_" + k] = v.astype(NP_WDT)

    lin = np.zeros((101, 4), np.float32)
    lin_w = np.asarray(lin_w, np.float32)
    linS_w = np.asarray(linS_w, np.float32)
    lin[0:100, 0] = lin_w[0, 0:100]
    lin[100, 0] = np.asarray(lin_b, np.float32).reshape(-1)[0]
    lin[0:100, 1] = lin_w[0, 100:200]
    lin[0:100, 2] = linS_w[0, 0:100]
    lin[100, 2] = np.asarray(linS_b, np.float32).reshape(-1)[0]
    lin[0:100, 3] = linS_w[0, 100:200]
    weights["lin"] = lin.astype(NP_WDT)

    in_maps = []
    for c in range(NCORES):
        xs = xt_p[c * BC:(c + 1) * BC]          # (64, 24, 8)
        xo = xo_p[c * BC:(c + 1) * BC]
        xt9 = np.ones((NV + 1, NF), np.float32)
        xt9[0:NV] = xs.transpose(2, 1, 0).reshape(NV, NF)   # [ch, t*64+b]
        xev = xo.transpose(2, 1, 0)[1:3].reshape(2, NF)     # [ch-1, t*64+b]
        m = dict(weights)
        m["xt9"] = xt9.astype(NP_WDT)
        m["xev"] = np.ascontiguousarray(xev.astype(np.float32))
        in_maps.append(m)
    return in_maps


# ----------------------------------------------------------------------------
# device kernel
# ----------------------------------------------------------------------------

def _declare(nc):
    d = {}
    d["xt9"] = nc.declare_dram_parameter("xt9", [NV + 1, NF], WDT, isOutput=False)
    d["xev"] = nc.declare_dram_parameter("xev", [2, NF], F32, isOutput=False)
    shapes = {}
    for pref in ["e", "s", "c"]:
        shapes[pref + "_wi0"] = [NV + 1, 800]
        shapes[pref + "_wh0"] = [100, 800]
        shapes[pref + "_wi1a"] = [101, 800]
        shapes[pref + "_wi1b"] = [100, 800]
        shapes[pref + "_wh1"] = [100, 800]
    for k in D_WNAMES:
        shapes["d_" + k] = [101 if k in ("wi0a", "wi1a") else 100, 800]
    for pref in ["ds", "dc"]:
        for k in DS_WNAMES:
            shapes[pref + "_" + k] = [101 if k in ("wi0a", "wi1a") else 100, 1600]
    shapes["lin"] = [101, 4]
    for k, shp in shapes.items():
        d[k] = nc.declare_dram_parameter(k, shp, WDT, isOutput=False)
    d["out"] = nc.declare_dram_parameter("out", [2 * FC, BC], F32, isOutput=True)
    return d


def _strided_blocks(big, off, nf):
    """AP over nf 64-wide blocks spaced STRIDE apart starting at col off."""
    return big[:, off:off + nf * STRIDE].rearrange(
        "p (f k) -> p f k", k=STRIDE)[:, :, 0:BC]


def _emit_macro_step(nc, work, psum, parts, c_tile, h_dest_fn, n_lo, n_hi, first):
    """One f-batched LSTM step over active cols [n_lo, n_hi).

    parts: list of (lhsT_fn, rhs_fn). lhsT_fn(gi) -> AP (K, 100).
    rhs_fn(c0, c1) -> list of (off, width, rhs_ap) covering [c0,c1) disjointly
    (off relative to c0). parts[0] must be a single full-width piece when
    len(parts) > 1. h_dest_fn(c0, c1) -> AP for the final h write.
    """
    single = len(parts) == 1
    for c0 in range(n_lo, n_hi, 512):
        c1 = min(c0 + 512, n_hi)
        w = c1 - c0
        pifo = psum.tile([100, 1536], F32, tag="ifo", bufs=2, name="pifo")
        pgg = psum.tile([100, 512], F32, tag="gg", bufs=2, name="pgg")
        for gi in range(4):
            out_ap = pifo[:, gi * 512: gi * 512 + w] if gi < 3 else pgg[:, 0:w]
            mms = []
            for pi, (lhsT_fn, rhs_fn) in enumerate(parts):
                for (off, ww, rhs_ap) in rhs_fn(c0, c1):
                    mms.append((lhsT_fn(gi), off, ww, rhs_ap, pi == 0))
            n = len(mms)
            for j, (lh, off, ww, rr, is_primary) in enumerate(mms):
                # column-disjoint pieces with no accumulation on top are each
                # their own start+stop group
                nc.tensor.matmul(out_ap[:, off:off + ww], lh, rr,
                                 start=(True if single else j == 0),
                                 stop=(True if single else j == n - 1))
        A = work.tile([100, 1536], BF16, tag="A", bufs=3, name="A")
        Ag = work.tile([100, 512], BF16, tag="Ag", bufs=3, name="Ag")
        if w == 512:
            nc.scalar.activation(A[:, 0:1536], pifo[:, 0:1536], AF.Sigmoid)
        else:
            src = pifo[:, 0:1536].rearrange("p (g k) -> p g k", k=512)[:, :, 0:w]
            dst = A[:, 0:1536].rearrange("p (g k) -> p g k", k=512)[:, :, 0:w]
            nc.scalar.activation(dst, src, AF.Sigmoid)
        nc.scalar.activation(Ag[:, 0:w], pgg[:, 0:w], AF.Tanh)
        cc = c_tile[:, c0:c1]
        if first:
            nc.vector.tensor_tensor(cc, A[:, 0:w], Ag[:, 0:w], ALU.mult)
        else:
            tig = work.tile([100, 512], BF16, tag="tig", bufs=3, name="tig")
            nc.vector.tensor_tensor(tig[:, 0:w], A[:, 0:w], Ag[:, 0:w], ALU.mult)
            tfc = work.tile([100, 512], F32, tag="tfc", bufs=2, name="tfc")
            nc.vector.tensor_tensor(tfc[:, 0:w], A[:, 512:512 + w], cc, ALU.mult)
            nc.vector.tensor_tensor(cc, tfc[:, 0:w], tig[:, 0:w], ALU.add)
        th = work.tile([100, 512], BF16, tag="th", bufs=3, name="th")
        nc.scalar.activation(th[:, 0:w], cc, AF.Tanh)
        nc.vector.tensor_tensor(h_dest_fn(c0, c1), A[:, 1024:1024 + w],
                                th[:, 0:w], ALU.mult)


def _emit_cell64(nc, work, psum, parts, nM, c_ap, h_out_ap, first,
                 psum_tag="gg"):
    """One N=64 LSTM cell. Gates in one PSUM tile, layout [i|f|o|g] (nM=4,
    H=100) or [ia ib fa fb oa ob ga gb] (nM=8, H=200).

    parts: list of (lhsT_ap, rhs_ap); each full width; chunk m slices
    lhsT[:, m*100:(m+1)*100]. parts[0] is the primary (start=True).
    """
    W = (nM // 4) * BC  # per-gate total width: 64 or 128
    pg = psum.tile([100, 512], F32, tag=psum_tag, bufs=2, name="pg64")
    n = len(parts)
    for m in range(nM):
        out_ap = pg[:, m * BC:(m + 1) * BC]
        for j, (lh, rr) in enumerate(parts):
            nc.tensor.matmul(out_ap, lh[:, m * 100:(m + 1) * 100], rr,
                             start=(j == 0), stop=(j == n - 1))
    Aa = work.tile([100, 512], BF16, tag="Ag", bufs=3, name="Aa")
    nsig = 3 * W
    nc.scalar.activation(Aa[:, 0:nsig], pg[:, 0:nsig], AF.Sigmoid)
    nc.scalar.activation(Aa[:, nsig:4 * W], pg[:, nsig:4 * W], AF.Tanh)
    i_s, f_s, o_s, g_s = (Aa[:, k * W:(k + 1) * W] for k in range(4))
    if first:
        nc.vector.tensor_tensor(c_ap, i_s, g_s, ALU.mult)
    else:
        tig = work.tile([100, 128], BF16, tag="tig64", bufs=3, name="tig64")
        nc.vector.tensor_tensor(tig[:, 0:W], i_s, g_s, ALU.mult)
        tfc = work.tile([100, 128], F32, tag="tfc64", bufs=3, name="tfc64")
        nc.vector.tensor_tensor(tfc[:, 0:W], f_s, c_ap, ALU.mult)
        nc.vector.tensor_tensor(c_ap, tfc[:, 0:W], tig[:, 0:W], ALU.add)
    th = work.tile([100, 128], BF16, tag="th64", bufs=3, name="th64")
    nc.scalar.activation(th[:, 0:W], c_ap, AF.Tanh)
    nc.vector.tensor_tensor(h_out_ap, o_s, th[:, 0:W], ALU.mult)


def _emit_shift_phase(nc, work, psum, sp, xt9, yf1, wts, mask_b, s0_out):
    """One shift_accumulate. wts: dict of weight tile APs (dir-major cols).
    Writes the masked 200-feature sum into s0_out (100, 128) bf16 [a|b]."""
    bigY = sp.tile([100, T * NF + STRIDE], BF16, tag="bigY", bufs=1, name="bigY")
    cL0 = sp.tile([100, NF], F32, tag="cx", bufs=1, name="cL0")
    cF = sp.tile([100, NF], F32, tag="cF", bufs=1, name="cF")
    outB = sp.tile([100, NF], BF16, tag="outB", bufs=1, name="outB")

    def wi0(gi):
        return wts["wi0"][:, 400 + gi * 100: 400 + (gi + 1) * 100]

    def wh0(gi):
        return wts["wh0"][:, 400 + gi * 100: 400 + (gi + 1) * 100]

    def wl1(name, d):
        return lambda gi: wts[name][:, d * 400 + gi * 100: d * 400 + (gi + 1) * 100]

    # --- bwd L0 over the f-batch (writes bigY step-major)
    for tt in range(T):
        S = (tt + 1) * BC

        def xrhs(c0, c1, S=S, tt=tt):
            out = []
            if c0 < S:
                e = min(c1, S)
                base = (T - 1 - tt) * BC
                out.append((0, e - c0, xt9[:, base + c0: base + e]))
            if c1 > S:
                s = max(c0, S)
                out.append((s - c0, c1 - s, xt9[:, s - S:c1 - S]))
            return out

        def hrhs(c0, c1, tt=tt):
            return [(0, c1 - c0, bigY[:, (tt - 1) * NF + c0:(tt - 1) * NF + c1])]

        if tt > 0:
            parts = [(wh0, hrhs), (wi0, xrhs)]
        else:
            parts = [(wi0, xrhs)]
        _emit_macro_step(nc, work, psum, parts, cL0,
                         lambda c0, c1, tt=tt: bigY[:, tt * NF + c0:tt * NF + c1],
                         0, NF, first=(tt == 0))

    # --- bwd L1 (shrinking suffix; cB reuses cL0's slot => starts after L0)
    cB = sp.tile([100, NF], F32, tag="cx", bufs=1, name="cB")
    hB_prev = None
    for tt in range(T):
        hB = sp.tile([100, NF], BF16, tag="hB", bufs=2, name="hB")

        def yb_rhs(c0, c1, tt=tt):
            return [(0, c1 - c0, bigY[:, tt * NF + c0:tt * NF + c1])]

        def yf_rhs(c0, c1, tt=tt):
            out = []
            S = (tt + 1) * BC
            if c0 < S:
                out.append((0, BC, yf1[:, (T - 1) * BC:T * BC]))
            if c1 > S:
                s = max(c0, S)
                out.append((s - c0, c1 - s, yf1[:, s - S:c1 - S]))
            return out

        parts = [(wl1("wi1b", 1), yb_rhs), (wl1("wi1a", 1), yf_rhs)]
        if tt > 0:
            def hh_rhs(c0, c1, hB_prev=hB_prev):
                return [(0, c1 - c0, hB_prev[:, c0:c1])]
            parts.append((wl1("wh1", 1), hh_rhs))
        _emit_macro_step(nc, work, psum, parts, cB,
                         lambda c0, c1, hB=hB: hB[:, c0:c1],
                         tt * BC, NF, first=(tt == 0))
        nc.vector.tensor_copy(outB[:, tt * BC:(tt + 1) * BC],
                              hB[:, tt * BC:(tt + 1) * BC])
        hB_prev = hB

    # --- fwd L1 (full 24 steps; needs all of bigY)
    hF_prev = None
    outF = None
    for tt in range(T):
        hF = sp.tile([100, NF], BF16, tag="hF", bufs=2, name="hF")

        def yf_rhs(c0, c1, tt=tt):
            nb = (c1 - c0) // BC
            return [(0, c1 - c0,
                     yf1[:, tt * BC:(tt + 1) * BC].unsqueeze(1)
                     .broadcast_to([101, nb, BC]))]

        def yb_rhs(c0, c1, tt=tt):
            out = []
            Sp = (tt + 1) * BC
            if c0 < Sp:
                e = min(c1, Sp)
                f0 = c0 // BC
                nf = (e - c0) // BC
                out.append((0, e - c0,
                            _strided_blocks(bigY, (T - 1 - tt) * NF + f0 * STRIDE,
                                            nf)))
            if c1 > Sp:
                s = max(c0, Sp)
                f0 = s // BC
                nf = (c1 - s) // BC
                off = f0 * STRIDE - (1 + tt) * NF
                out.append((s - c0, c1 - s, _strided_blocks(bigY, off, nf)))
            return out

        parts = [(wl1("wi1a", 0), yf_rhs), (wl1("wi1b", 0), yb_rhs)]
        if tt > 0:
            def hh_rhs(c0, c1, hF_prev=hF_prev):
                return [(0, c1 - c0, hF_prev[:, c0:c1])]
            parts.append((wl1("wh1", 0), hh_rhs))
        _emit_macro_step(nc, work, psum, parts, cF,
                         lambda c0, c1, hF=hF: hF[:, c0:c1],
                         0, NF, first=(tt == 0))
        hF_prev = hF
        if tt == T - 1:
            outF = hF

    # --- masked accumulation: s0 = sum_f mask[:, f] * [outF; outB]
    for half, src in [(0, outF), (1, outB)]:
        tmp = work.tile([100, NF], F32, tag="redtmp", bufs=1, name="redtmp")
        nc.vector.tensor_tensor(tmp[:], src[:], mask_b[:], ALU.mult)
        red = work.tile([100, BC], F32, tag="red64", bufs=2, name="red64")
        nc.vector.tensor_reduce(
            red[:].unsqueeze(2),
            tmp[:].rearrange("p (f k) -> p k f", f=T),
            mybir.AxisListType.X, ALU.add)
        nc.vector.tensor_copy(s0_out[:, half * BC:(half + 1) * BC], red[:])


def build_kernel():
    nc = bacc.Bacc("TRN2", target_bir_lowering=False, debug=False)
    d = _declare(nc)
    enc_wnames = [f"{p}_{k}" for p in ("e", "s", "c") for k in ENC_WNAMES]
    dec_wnames = (["d_" + k for k in D_WNAMES]
                  + [f"{p}_{k}" for p in ("ds", "dc") for k in DS_WNAMES]
                  + ["lin"])
    with TileContext(nc) as tc:
        with (
            tc.tile_pool(name="const", bufs=1) as const,
            tc.tile_pool(name="persist", bufs=1) as persist,
            tc.tile_pool(name="work", bufs=1) as work,
        ):
            # ---- load encoder-side constants
            wt = {}
            for name in enc_wnames:
                dh = d[name]
                t = const.tile(list(dh.shape), WDT, tag="w_" + name, bufs=1,
                               name="w_" + name)
                nc.sync.dma_start(t[:], dh[:])
                wt[name] = t
            xt9 = const.tile([NV + 1, NF], WDT, tag="xt9t", bufs=1, name="xt9t")
            nc.sync.dma_start(xt9[:], d["xt9"][:])

            def P(shape, dt, name):
                return persist.tile(shape, dt, tag=name, bufs=1, name=name)

            yf1S = P([101, NF], BF16, "yf1S")
            yf1C = P([101, NF], BF16, "yf1C")
            yf1E = P([101, NF], BF16, "yf1E")
            bigYE = P([100, NF], BF16, "bigYE")
            s0_bf = P([100, 128], BF16, "s0_bf")
            sc0_bf = P([100, 128], BF16, "sc0_bf")
            enc_bf = P([100, 128], BF16, "enc_bf")
            maskS_b = P([100, NF], BF16, "maskS_b")
            maskC_b = P([100, NF], BF16, "maskC_b")
            mI = P([100, 128], BF16, "mI")
            mC = P([100, 128], BF16, "mC")
            mBoth = P([1, BC], F32, "mBoth")
            OUT = P([1, 2 * FC * BC], F32, "OUT")
            for yf in (yf1S, yf1C, yf1E):
                # rows 0:100 are fully overwritten by the L0 runs; only the
                # ones-row (bias rhs) must survive
                nc.vector.memset(yf[:], 1.0)

            with tc.tile_pool(name="psum_main", bufs=1, space="PSUM") as psum:
                # ---- masks from xev
                with tc.tile_pool(name="setup", bufs=1) as setup:
                    xev = setup.tile([2, NF], F32, tag="xev_t", bufs=1,
                                     name="xev_t")
                    nc.sync.dma_start(xev[:], d["xev"][:])
                    rows = setup.tile([2, NF], F32, tag="rows", bufs=1,
                                      name="rows")
                    nc.vector.tensor_scalar(rows[:], xev[:], 0.0, None, ALU.is_gt)
                    ones1 = setup.tile([1, 100], F32, tag="ones1", bufs=1,
                                       name="ones1")
                    nc.any.memset(ones1[:], 1.0)
                    # f-batched mask broadcasts (100, NF)
                    for r, dst in [(0, maskS_b), (1, maskC_b)]:
                        for c0 in range(0, NF, 512):
                            pm = psum.tile([100, 512], F32, tag="gg", bufs=2,
                                           name="pmask")
                            nc.tensor.matmul(pm[:], ones1[:],
                                             rows[r:r + 1, c0:c0 + 512],
                                             start=True, stop=True)
                            nc.scalar.copy(dst[:, c0:c0 + 512], pm[:])
                    # per-sample masks (any event over t)
                    sm = setup.tile([2, BC], F32, tag="sm", bufs=1, name="sm")
                    nc.vector.tensor_reduce(
                        sm[:].unsqueeze(2),
                        rows[:].rearrange("p (f k) -> p k f", f=T),
                        mybir.AxisListType.X, ALU.add)
                    smr = setup.tile([2, BC], F32, tag="smr", bufs=1, name="smr")
                    nc.vector.tensor_scalar(smr[:], sm[:], 0.0, None, ALU.is_gt)
                    both = setup.tile([1, BC], F32, tag="both", bufs=1,
                                      name="both")
                    nc.vector.tensor_tensor(both[:], sm[0:1, :], sm[1:2, :],
                                            ALU.add)
                    nc.vector.tensor_scalar(mBoth[:], both[:], 0.0, None,
                                            ALU.is_gt)
                    for r, dst in [(0, mI), (1, mC)]:
                        pm = psum.tile([100, 512], F32, tag="gg", bufs=2,
                                       name="pmask2")
                        nc.tensor.matmul(pm[:, 0:BC], ones1[:], smr[r:r + 1, :],
                                         start=True, stop=True)
                        nc.scalar.copy(dst[:, 0:BC], pm[:, 0:BC])
                        nc.vector.tensor_copy(dst[:, BC:128], dst[:, 0:BC])

                # ---- prologue: fwd-L0 runs for S, C, E (+ E bwd-L0), N=64
                def run_l0_64(wi0_t, wh0_t, d0, rhs_fn, h_dest_fn, ctag):
                    c64 = persist.tile([100, BC], F32, tag=ctag, bufs=1,
                                       name=ctag)
                    h_prev = None
                    for t_ in range(T):
                        parts = [(wi0_t[:, d0 * 400:(d0 + 1) * 400], rhs_fn(t_))]
                        if t_ > 0:
                            parts.append((wh0_t[:, d0 * 400:(d0 + 1) * 400],
                                          h_prev))
                        _emit_cell64(nc, work, psum, parts, 4, c64[:],
                                     h_dest_fn(t_), first=(t_ == 0))
                        h_prev = h_dest_fn(t_)

                run_l0_64(wt["s_wi0"], wt["s_wh0"], 0,
                          lambda t_: xt9[:, t_ * BC:(t_ + 1) * BC],
                          lambda t_: yf1S[0:100, t_ * BC:(t_ + 1) * BC], "c64s")
                run_l0_64(wt["c_wi0"], wt["c_wh0"], 0,
                          lambda t_: xt9[:, t_ * BC:(t_ + 1) * BC],
                          lambda t_: yf1C[0:100, t_ * BC:(t_ + 1) * BC], "c64c")
                run_l0_64(wt["e_wi0"], wt["e_wh0"], 0,
                          lambda t_: xt9[:, t_ * BC:(t_ + 1) * BC],
                          lambda t_: yf1E[0:100, t_ * BC:(t_ + 1) * BC], "c64e")
                run_l0_64(wt["e_wi0"], wt["e_wh0"], 1,
                          lambda t_: xt9[:, (T - 1 - t_) * BC:(T - t_) * BC],
                          lambda t_: bigYE[:, t_ * BC:(t_ + 1) * BC], "c64eb")

                # ---- shift phases (the heavy part)
                s_wts = {k[2:]: v for k, v in wt.items() if k.startswith("s_")}
                c_wts = {k[2:]: v for k, v in wt.items() if k.startswith("c_")}
                with tc.tile_pool(name="shiftS", bufs=1) as sp:
                    _emit_shift_phase(nc, work, psum, sp, xt9, yf1S, s_wts,
                                      maskS_b, s0_bf)
                with tc.tile_pool(name="shiftC", bufs=1) as sp:
                    _emit_shift_phase(nc, work, psum, sp, xt9, yf1C, c_wts,
                                      maskC_b, sc0_bf)

                # ---- encoder fwd L1 + 1-step bwd L1
                cEf1 = P([100, BC], F32, "cEf1")
                hE_prev = None
                for t_ in range(T):
                    hE = work.tile([100, BC], BF16, tag="hEf1", bufs=2,
                                   name="hEf1")
                    parts = [(wt["e_wi1a"][:, 0:400],
                              yf1E[:, t_ * BC:(t_ + 1) * BC]),
                             (wt["e_wi1b"][:, 0:400],
                              bigYE[:, (T - 1 - t_) * BC:(T - t_) * BC])]
                    if t_ > 0:
                        parts.append((wt["e_wh1"][:, 0:400], hE_prev[:]))
                    dest = enc_bf[:, 0:BC] if t_ == T - 1 else hE[:]
                    _emit_cell64(nc, work, psum, parts, 4, cEf1[:], dest,
                                 first=(t_ == 0))
                    hE_prev = hE
                cEb1 = P([100, BC], F32, "cEb1")
                parts = [(wt["e_wi1a"][:, 400:800], yf1E[:, (T - 1) * BC:T * BC]),
                         (wt["e_wi1b"][:, 400:800], bigYE[:, 0:BC])]
                _emit_cell64(nc, work, psum, parts, 4, cEb1[:],
                             enc_bf[:, BC:128], first=True)

            # ---- decoder
            with (
                tc.tile_pool(name="psum_dec", bufs=1, space="PSUM") as psd,
                tc.tile_pool(name="decp", bufs=1) as dp,
            ):
                for name in dec_wnames:
                    dh = d[name]
                    t = dp.tile(list(dh.shape), WDT, tag="w_" + name, bufs=1,
                                name="w_" + name)
                    nc.sync.dma_start(t[:], dh[:])
                    wt[name] = t

                def st_h(name, w):
                    ts = []
                    for k in range(2):
                        t_ = dp.tile([101, w], BF16, tag=f"{name}_{k}", bufs=1,
                                     name=f"{name}_{k}")
                        nc.vector.memset(t_[:], 1.0)
                        nc.vector.memset(t_[0:100, :], 0.0)
                        ts.append(t_)
                    return ts

                def st_c(name, w):
                    return [dp.tile([100, w], F32, tag=f"{name}c_{k}", bufs=1,
                                    name=f"{name}c_{k}") for k in range(2)]

                y_t = st_h("y", 128)
                ssum = dp.tile([101, 128], BF16, tag="ssum", bufs=1, name="ssum")
                nc.vector.memset(ssum[:], 1.0)
                states = {}
                for dec, w in [("d", 64), ("ds", 128), ("dc", 128)]:
                    for lay in ("0", "1"):
                        for dr in ("f", "b"):
                            states[f"{dec}h{lay}{dr}"] = st_h(f"{dec}h{lay}{dr}", w)
                            states[f"{dec}c{lay}{dr}"] = st_c(f"{dec}c{lay}{dr}", w)
                nc.vector.tensor_copy(y_t[1][0:100, :], enc_bf[:])

                def dec_cell(dec, lay, dr, k, xparts, nM, first):
                    cur, prev = k % 2, (k + 1) % 2
                    h_prev = states[f"{dec}h{lay}{dr}"][prev]
                    h_out = states[f"{dec}h{lay}{dr}"][cur]
                    c_t = states[f"{dec}c{lay}{dr}"][0]
                    di = 0 if dr == "f" else 1
                    parts = list(xparts(di))
                    if not first:
                        if dec == "d":
                            parts.append((wt[f"d_wh{lay}"][:, di * 400:(di + 1) * 400],
                                          h_prev[0:100, :]))
                        else:
                            parts.append((wt[f"{dec}_wh{lay}a"][:, di * 800:(di + 1) * 800],
                                          h_prev[0:100, 0:BC]))
                            parts.append((wt[f"{dec}_wh{lay}b"][:, di * 800:(di + 1) * 800],
                                          h_prev[0:100, BC:128]))
                    _emit_cell64(nc, work, psd, parts, nM, c_t[:],
                                 h_out[0:100, :], first, psum_tag="cell")
                    return h_out

                for k in range(FC):
                    cur, prev = k % 2, (k + 1) % 2
                    y_prev = y_t[prev]
                    first = (k == 0)
                    if k == 0:
                        s_a, s_b = s0_bf[:, 0:BC], s0_bf[:, BC:128]
                        sc_a, sc_b = sc0_bf[:, 0:BC], sc0_bf[:, BC:128]
                    else:
                        s_a = states["dsh1b"][prev][0:100, 0:BC]
                        s_b = states["dsh1b"][prev][0:100, BC:128]
                        sc_a = states["dch1b"][prev][0:100, 0:BC]
                        sc_b = states["dch1b"][prev][0:100, BC:128]

                    def d_x0(di, y_prev=y_prev):
                        return [(wt["d_wi0a"][:, di * 400:(di + 1) * 400],
                                 y_prev[0:101, 0:BC]),
                                (wt["d_wi0b"][:, di * 400:(di + 1) * 400],
                                 y_prev[0:100, BC:128])]
                    h0f = dec_cell("d", "0", "f", k, d_x0, 4, first)
                    h0b = dec_cell("d", "0", "b", k, d_x0, 4, first)

                    def d_x1(di, h0f=h0f, h0b=h0b):
                        return [(wt["d_wi1a"][:, di * 400:(di + 1) * 400],
                                 h0f[0:101, :]),
                                (wt["d_wi1b"][:, di * 400:(di + 1) * 400],
                                 h0b[0:100, :])]
                    yd_a = dec_cell("d", "1", "f", k, d_x1, 4, first)
                    yd_b = dec_cell("d", "1", "b", k, d_x1, 4, first)

                    outs = {}
                    for dec, (sa, sb) in [("ds", (s_a, s_b)),
                                          ("dc", (sc_a, sc_b))]:
                        def s_x0(di, dec=dec, sa=sa, sb=sb, y_prev=y_prev):
                            return [
                                (wt[f"{dec}_wi0a"][:, di * 800:(di + 1) * 800],
                                 y_prev[0:101, 0:BC]),
                                (wt[f"{dec}_wi0b"][:, di * 800:(di + 1) * 800],
                                 y_prev[0:100, BC:128]),
                                (wt[f"{dec}_wi0c"][:, di * 800:(di + 1) * 800], sa),
                                (wt[f"{dec}_wi0d"][:, di * 800:(di + 1) * 800], sb),
                            ]
                        g0f = dec_cell(dec, "0", "f", k, s_x0, 8, first)
                        g0b = dec_cell(dec, "0", "b", k, s_x0, 8, first)

                        def s_x1(di, dec=dec, g0f=g0f, g0b=g0b):
                            return [
                                (wt[f"{dec}_wi1a"][:, di * 800:(di + 1) * 800],
                                 g0f[0:101, 0:BC]),
                                (wt[f"{dec}_wi1b"][:, di * 800:(di + 1) * 800],
                                 g0f[0:100, BC:128]),
                                (wt[f"{dec}_wi1c"][:, di * 800:(di + 1) * 800],
                                 g0b[0:100, 0:BC]),
                                (wt[f"{dec}_wi1d"][:, di * 800:(di + 1) * 800],
                                 g0b[0:100, BC:128]),
                            ]
                        outs[dec + "f"] = dec_cell(dec, "1", "f", k, s_x1, 8,
                                                   first)
                        outs[dec + "b"] = dec_cell(dec, "1", "b", k, s_x1, 8,
                                                   first)

                    # --- yx = yd - mI*relu(ySf) + mC*relu(ySCf)
                    y_new = y_t[cur]
                    r1 = work.tile([100, 128], BF16, tag="r1", bufs=2, name="r1")
                    nc.vector.tensor_scalar(r1[:], outs["dsf"][0:100, :], 0.0,
                                            None, ALU.max)
                    rm1 = work.tile([100, 128], BF16, tag="rm1", bufs=2,
                                    name="rm1")
                    nc.vector.tensor_tensor(rm1[:], r1[:], mI[:], ALU.mult)
                    nc.vector.tensor_tensor(y_new[0:100, 0:BC], yd_a[0:100, :],
                                            rm1[:, 0:BC], ALU.subtract)
                    nc.vector.tensor_tensor(y_new[0:100, BC:128], yd_b[0:100, :],
                                            rm1[:, BC:128], ALU.subtract)
                    r2 = work.tile([100, 128], BF16, tag="r1", bufs=2, name="r2")
                    nc.vector.tensor_scalar(r2[:], outs["dcf"][0:100, :], 0.0,
                                            None, ALU.max)
                    rm2 = work.tile([100, 128], BF16, tag="rm1", bufs=2,
                                    name="rm2")
                    nc.vector.tensor_tensor(rm2[:], r2[:], mC[:], ALU.mult)
                    nc.vector.tensor_tensor(y_new[0:100, 0:BC],
                                            y_new[0:100, 0:BC],
                                            rm2[:, 0:BC], ALU.add)
                    nc.vector.tensor_tensor(y_new[0:100, BC:128],
                                            y_new[0:100, BC:128],
                                            rm2[:, BC:128], ALU.add)

                    # --- out_f = yx @ lin_w.T + lin_b
                    pl = psd.tile([1, BC], F32, tag="lin", bufs=2, name="pl")
                    nc.tensor.matmul(pl[:], wt["lin"][:, 0:1],
                                     y_new[0:101, 0:BC], start=True, stop=False)
                    nc.tensor.matmul(pl[:], wt["lin"][0:100, 1:2],
                                     y_new[0:100, BC:128], start=False, stop=True)
                    nc.scalar.copy(OUT[:, k * BC:(k + 1) * BC], pl[:])

                    # --- outS = mBoth * ((mI*ySb + mC*ySCb) @ linS_w.T + linS_b)
                    t1 = work.tile([100, 128], BF16, tag="r1", bufs=2, name="t1")
                    nc.vector.tensor_tensor(t1[:], outs["dsb"][0:100, :], mI[:],
                                            ALU.mult)
                    t2 = work.tile([100, 128], BF16, tag="rm1", bufs=2, name="t2")
                    nc.vector.tensor_tensor(t2[:], outs["dcb"][0:100, :], mC[:],
                                            ALU.mult)
                    nc.vector.tensor_tensor(ssum[0:100, :], t1[:], t2[:], ALU.add)
                    pl2 = psd.tile([1, BC], F32, tag="lin", bufs=2, name="pl2")
                    nc.tensor.matmul(pl2[:], wt["lin"][:, 2:3],
                                     ssum[0:101, 0:BC], start=True, stop=False)
                    nc.tensor.matmul(pl2[:], wt["lin"][0:100, 3:4],
                                     ssum[0:100, BC:128], start=False, stop=True)
                    rowS = work.tile([1, BC], F32, tag="rowS", bufs=2,
                                     name="rowS")
                    nc.scalar.copy(rowS[:], pl2[:])
                    nc.vector.tensor_tensor(OUT[:, (FC + k) * BC:(FC + k + 1) * BC],
                                            rowS[:], mBoth[:], ALU.mult)

                nc.sync.dma_start(d["out"][:], OUT[:])
    return nc


_CACHED = None


def _install_ntff_shim():
    """Register a minimal antenv.axon_hooks so trace=True works under axon
    (only used when KERNEL_TRACE=1; the plain run never needs it)."""
    import contextlib
    import ctypes
    import types
    import glob

    hook = None
    cands = glob.glob("/opt/axon/libaxon_pjrt.so") + glob.glob(
        "/root/.axon_site/**/libaxon_pjrt.so", recursive=True)
    for so_path in cands:
        try:
            lib = ctypes.CDLL(so_path)
        except OSError:
            continue
        if not hasattr(lib, "axon_start_nrt_profile"):
            continue
        lib.axon_start_nrt_profile.argtypes = [
            ctypes.POINTER(ctypes.c_int64), ctypes.c_size_t]
        lib.axon_start_nrt_profile.restype = ctypes.c_int64
        lib.axon_stop_nrt_profile.argtypes = [ctypes.c_char_p]
        lib.axon_stop_nrt_profile.restype = ctypes.c_int64

        @contextlib.contextmanager
        def _hook(output_dir, device_ids, lib=lib):
            import jax
            jax.devices()
            if device_ids:
                ids = (ctypes.c_int64 * len(device_ids))(*device_ids)
                rc = lib.axon_start_nrt_profile(ids, len(device_ids))
            else:
                rc = lib.axon_start_nrt_profile(None, 0)
            if rc != 0:
                raise RuntimeError(f"axon_start_nrt_profile rc={rc}")
            try:
                yield
            finally:
                n = lib.axon_stop_nrt_profile(str(output_dir).encode())
                print(f"ntff profile: {n} file(s) -> {output_dir}",
                      file=sys.stderr)
        hook = _hook
        break
    mod = types.ModuleType("antenv.axon_hooks")
    mod.get_axon_ntff_profile_hook = lambda: hook
    mod.set_axon_ntff_profile_hook = lambda h: None
    import antenv
    antenv.axon_hooks = mod
    sys.modules["antenv.axon_hooks"] = mod


def kernel(**inputs):
    global _CACHED
    if _CACHED is None:
        _CACHED = build_kernel()
    nc = _CACHED
    in_maps = _prep_inputs(**inputs)
    trace = os.environ.get("KERNEL_TRACE", "0") == "1"
    if trace:
        _install_ntff_shim()
        r = run_bass_kernel_spmd(nc, in_maps, list(range(NCORES)), trace=True)
        print(f"HW exec time: {r.exec_time_ns} ns")
        res = r.results
    else:
        res = run_bass_kernel_spmd(nc, in_maps, list(range(NCORES))).results
    outer = np.zeros((BP, FC), np.float32)
    outS = np.zeros((BP, FC), np.float32)
    for c in range(NCORES):
        o = res[c]["out"]  # (24, 64)
        outer[c * BC:(c + 1) * BC] = o[0:FC].T
        outS[c * BC:(c + 1) * BC] = o[FC:2 * FC].T
    return outer[:B], outS[:B]


# revision 8
# speedup vs baseline: 1.1433x; 1.1433x over previous
"""Trainium2 Bass kernel for nn_Block_66073776882206 (ragged_sequence).

Strategy
--------
Pure data parallelism over the batch: pad 500 -> 512, shard 64 samples per
NeuronCore across 8 cores, replicate all weights. Everything on-device is
feature-major: SBUF tiles are (features<=128, batch-cols), LSTM cell matmuls
use lhsT = weight chunks (K=feat_in, M=gate_rows), rhs = activations
(K, N=batch-cols), PSUM out (gate_rows, batch-cols).

The heavy part (shift_accumulate) runs, for each backcast step f in [0,24),
a two-segment bidirectional 2-layer LSTM over the length-24 sequence. The
f-axis is batched into the matmul N dimension: N = 24 f-blocks x 64 batch =
1536 columns. The permutation perm_f[t] = (f-1-t if t<f else 23-(t-f)) is
linear in f with slope 1 in both branches, so every permuted read/write
reduces to a contiguous slice or a stride-1600 access pattern on a
step-major store -- no gather DMAs. The backward L1 run for block f only
needs steps 0..f (its consumed output is the processing-step-f entry), so
it runs on a shrinking column suffix (~52% of the full work).

Gate order is host-reordered to [i, f, o, g] so one ACT instruction applies
sigmoid across i,f,o and one applies tanh to g. Biases are folded into the
x-side matmul via an appended ones-row on the rhs / bias-row on the lhsT.

Matmul inputs are bf16; PSUM accumulation and the cell state c stay fp32.
"""
import sys
import os

sys.path.insert(0, "/opt/trn_rl_repo")

import numpy as np
import ml_dtypes

import concourse.bass as bass
import concourse.bacc as bacc
import concourse.mybir as mybir
from concourse.tile import TileContext
from concourse.bass_utils import run_bass_kernel_spmd

F32 = mybir.dt.float32
BF16 = mybir.dt.bfloat16
AF = mybir.ActivationFunctionType
ALU = mybir.AluOpType

NV = 8
H = 100          # UNITS
T = 24           # BACKLEN
FC = 12          # FORECAST
B = 500
NCORES = 8
BC = 64          # batch per core (padded)
BP = NCORES * BC  # 512
NF = T * BC      # 1536 f-batched columns
STRIDE = NF + BC  # 1600: f-block stride between (step, block) diagonals
H2 = 2 * H

WDT = BF16
NP_WDT = ml_dtypes.bfloat16

GATE_PERM = [0, 1, 3, 2]  # reference order [i, f, g, o] -> ours [i, f, o, g]

ENC_WNAMES = ["wi0", "wh0", "wfus0", "wi1a", "wi1b", "wh1"]
D_WNAMES = ["wi0a", "wi0b", "wh0", "wi1a", "wi1b", "wh1"]
DS_WNAMES = ["wi0a", "wi0b", "wi0c", "wi0d", "wh0a", "wh0b",
             "wi1a", "wi1b", "wi1c", "wi1d", "wh1a", "wh1b"]


# ----------------------------------------------------------------------------
# host-side weight prep
# ----------------------------------------------------------------------------

def _reorder(M):
    """(4H, X) -> rows gate-reordered to [i, f, o, g]."""
    Hq = M.shape[0] // 4
    return M.reshape(4, Hq, -1)[GATE_PERM].reshape(4 * Hq, -1)


def _wi_aug(Wi, b, zero_ch=()):
    """W_ih (4H, I), b (4H,) -> lhsT (I+1, 4H): rows = input features + a
    bias row (applied via the ones-row in rhs); cols = reordered gate rows."""
    W = np.array(Wi, dtype=np.float32).copy()
    for c in zero_ch:
        W[:, c] = 0.0
    W = _reorder(W)
    bb = _reorder(np.asarray(b, np.float32).reshape(-1, 1))
    return np.concatenate([W.T, bb.T], axis=0)


def _wh(Wh):
    return np.ascontiguousarray(_reorder(np.asarray(Wh, np.float32)).T)


def _enc_pack(p, zero_ch):
    """2-layer bidir LSTM with H=100, I=8 -> dict of (K, 2*400) arrays."""
    out = {k: [] for k in ENC_WNAMES}
    for d in range(2):
        full0 = _wi_aug(p["W_ih0"][d], p["b0"][d], zero_ch)      # (9, 400)
        out["wi0"].append(full0)
        wh0 = _wh(p["W_hh0"][d])                                  # (100, 400)
        out["wh0"].append(wh0)
        out["wfus0"].append(np.concatenate([wh0, full0], 0))      # (109, 400)
        full1 = _wi_aug(p["W_ih1"][d], p["b1"][d])                # (201, 400)
        out["wi1a"].append(np.concatenate([full1[0:H], full1[2 * H:2 * H + 1]], 0))
        out["wi1b"].append(full1[H:2 * H])
        out["wh1"].append(_wh(p["W_hh1"][d]))
    return {k: np.concatenate(v, 1) for k, v in out.items()}


def _dec_pack(p):
    """p_dec: H=100, I=200."""
    out = {k: [] for k in D_WNAMES}
    for d in range(2):
        full0 = _wi_aug(p["W_ih0"][d], p["b0"][d])                # (201, 400)
        out["wi0a"].append(np.concatenate([full0[0:H], full0[2 * H:2 * H + 1]], 0))
        out["wi0b"].append(full0[H:2 * H])
        out["wh0"].append(_wh(p["W_hh0"][d]))
        full1 = _wi_aug(p["W_ih1"][d], p["b1"][d])
        out["wi1a"].append(np.concatenate([full1[0:H], full1[2 * H:2 * H + 1]], 0))
        out["wi1b"].append(full1[H:2 * H])
        out["wh1"].append(_wh(p["W_hh1"][d]))
    return {k: np.concatenate(v, 1) for k, v in out.items()}


def _decS_pack(p):
    """p_decS / p_decSC: H=200, I=400, 4H=800. K-chunks of 100 rows."""
    out = {k: [] for k in DS_WNAMES}
    for d in range(2):
        full0 = _wi_aug(p["W_ih0"][d], p["b0"][d])                # (401, 800)
        out["wi0a"].append(np.concatenate([full0[0:100], full0[400:401]], 0))
        out["wi0b"].append(full0[100:200])
        out["wi0c"].append(full0[200:300])
        out["wi0d"].append(full0[300:400])
        w0 = _wh(p["W_hh0"][d])                                    # (200, 800)
        out["wh0a"].append(w0[0:100])
        out["wh0b"].append(w0[100:200])
        full1 = _wi_aug(p["W_ih1"][d], p["b1"][d])
        out["wi1a"].append(np.concatenate([full1[0:100], full1[400:401]], 0))
        out["wi1b"].append(full1[100:200])
        out["wi1c"].append(full1[200:300])
        out["wi1d"].append(full1[300:400])
        w1 = _wh(p["W_hh1"][d])
        out["wh1a"].append(w1[0:100])
        out["wh1b"].append(w1[100:200])
    return {k: np.concatenate(v, 1) for k, v in out.items()}  # (100/101, 1600)


def _prep_inputs(xt, xorig, p_lstm, p_dec, p_lstmS, p_decS, p_lstmSC, p_decSC,
                 lin_w, lin_b, linS_w, linS_b):
    """Build the per-core input maps. Weights replicated; x sharded."""
    xt = np.asarray(xt, np.float32)
    xorig = np.asarray(xorig, np.float32)
    xt_p = np.zeros((BP, T, NV), np.float32)
    xo_p = np.zeros((BP, T, NV), np.float32)
    xt_p[:B] = xt
    xo_p[:B] = xorig

    weights = {}
    for pref, pk, zc in [("e", p_lstm, (1, 2)), ("s", p_lstmS, (2,)),
                         ("c", p_lstmSC, (1,))]:
        for k, v in _enc_pack(pk, zc).items():
            weights[pref + "_" + k] = v.astype(NP_WDT)
    for k, v in _dec_pack(p_dec).items():
        weights["d_" + k] = v.astype(NP_WDT)
    for k, v in _decS_pack(p_decS).items():
        weights["ds_" + k] = v.astype(NP_WDT)
    for k, v in _decS_pack(p_decSC).items():
        weights["dc_" + k] = v.astype(NP_WDT)

    lin = np.zeros((101, 4), np.float32)
    lin_w = np.asarray(lin_w, np.float32)
    linS_w = np.asarray(linS_w, np.float32)
    lin[0:100, 0] = lin_w[0, 0:100]
    lin[100, 0] = np.asarray(lin_b, np.float32).reshape(-1)[0]
    lin[0:100, 1] = lin_w[0, 100:200]
    lin[0:100, 2] = linS_w[0, 0:100]
    lin[100, 2] = np.asarray(linS_b, np.float32).reshape(-1)[0]
    lin[0:100, 3] = linS_w[0, 100:200]
    weights["lin"] = lin.astype(NP_WDT)

    in_maps = []
    for c in range(NCORES):
        xs = xt_p[c * BC:(c + 1) * BC]          # (64, 24, 8)
        xo = xo_p[c * BC:(c + 1) * BC]
        xt9 = np.ones((NV + 1, NF), np.float32)
        xt9[0:NV] = xs.transpose(2, 1, 0).reshape(NV, NF)   # [ch, t*64+b]
        xev = xo.transpose(2, 1, 0)[1:3].reshape(2, NF)     # [ch-1, t*64+b]
        m = dict(weights)
        m["xt9"] = xt9.astype(NP_WDT)
        m["xev"] = np.ascontiguousarray(xev.astype(np.float32))
        in_maps.append(m)
    return in_maps


# ----------------------------------------------------------------------------
# device kernel
# ----------------------------------------------------------------------------

def _declare(nc):
    d = {}
    d["xt9"] = nc.declare_dram_parameter("xt9", [NV + 1, NF], WDT, isOutput=False)
    d["xev"] = nc.declare_dram_parameter("xev", [2, NF], F32, isOutput=False)
    shapes = {}
    for pref in ["e", "s", "c"]:
        shapes[pref + "_wi0"] = [NV + 1, 800]
        shapes[pref + "_wh0"] = [100, 800]
        shapes[pref + "_wfus0"] = [100 + NV + 1, 800]
        shapes[pref + "_wi1a"] = [101, 800]
        shapes[pref + "_wi1b"] = [100, 800]
        shapes[pref + "_wh1"] = [100, 800]
    for k in D_WNAMES:
        shapes["d_" + k] = [101 if k in ("wi0a", "wi1a") else 100, 800]
    for pref in ["ds", "dc"]:
        for k in DS_WNAMES:
            shapes[pref + "_" + k] = [101 if k in ("wi0a", "wi1a") else 100, 1600]
    shapes["lin"] = [101, 4]
    for k, shp in shapes.items():
        d[k] = nc.declare_dram_parameter(k, shp, WDT, isOutput=False)
    d["out"] = nc.declare_dram_parameter("out", [2 * FC, BC], F32, isOutput=True)
    return d


def _strided_blocks(big, off, nf):
    """AP over nf 64-wide blocks spaced STRIDE apart starting at col off."""
    return big[:, off:off + nf * STRIDE].rearrange(
        "p (f k) -> p f k", k=STRIDE)[:, :, 0:BC]


def _emit_macro_step(nc, work, psum, parts, c_tile, h_dest_fn, n_lo, n_hi, first):
    """One f-batched LSTM step over active cols [n_lo, n_hi).

    parts: list of (lhsT_fn, rhs_fn). lhsT_fn(gi) -> AP (K, 100).
    rhs_fn(c0, c1) -> list of (off, width, rhs_ap) covering [c0,c1) disjointly
    (off relative to c0). parts[0] must be a single full-width piece when
    len(parts) > 1. h_dest_fn(c0, c1) -> AP for the final h write.
    """
    single = len(parts) == 1
    for c0 in range(n_lo, n_hi, 512):
        c1 = min(c0 + 512, n_hi)
        w = c1 - c0
        pifo = psum.tile([100, 1536], F32, tag="ifo", bufs=2, name="pifo")
        pgg = psum.tile([100, 512], F32, tag="gg", bufs=2, name="pgg")
        for gi in range(4):
            out_ap = pifo[:, gi * 512: gi * 512 + w] if gi < 3 else pgg[:, 0:w]
            mms = []
            for pi, (lhsT_fn, rhs_fn) in enumerate(parts):
                for (off, ww, rhs_ap) in rhs_fn(c0, c1):
                    mms.append((lhsT_fn(gi), off, ww, rhs_ap, pi == 0))
            n = len(mms)
            for j, (lh, off, ww, rr, is_primary) in enumerate(mms):
                # column-disjoint pieces with no accumulation on top are each
                # their own start+stop group
                nc.tensor.matmul(out_ap[:, off:off + ww], lh, rr,
                                 start=(True if single else j == 0),
                                 stop=(True if single else j == n - 1))
        A = work.tile([100, 1536], BF16, tag="A", bufs=3, name="A")
        Ag = work.tile([100, 512], BF16, tag="Ag", bufs=3, name="Ag")
        if w == 512:
            nc.scalar.activation(A[:, 0:1536], pifo[:, 0:1536], AF.Sigmoid)
        else:
            src = pifo[:, 0:1536].rearrange("p (g k) -> p g k", k=512)[:, :, 0:w]
            dst = A[:, 0:1536].rearrange("p (g k) -> p g k", k=512)[:, :, 0:w]
            nc.scalar.activation(dst, src, AF.Sigmoid)
        nc.scalar.activation(Ag[:, 0:w], pgg[:, 0:w], AF.Tanh)
        cc = c_tile[:, c0:c1]
        if first:
            nc.vector.tensor_tensor(cc, A[:, 0:w], Ag[:, 0:w], ALU.mult)
        else:
            tig = work.tile([100, 512], BF16, tag="tig", bufs=3, name="tig")
            nc.vector.tensor_tensor(tig[:, 0:w], A[:, 0:w], Ag[:, 0:w], ALU.mult)
            tfc = work.tile([100, 512], F32, tag="tfc", bufs=2, name="tfc")
            nc.vector.tensor_tensor(tfc[:, 0:w], A[:, 512:512 + w], cc, ALU.mult)
            nc.vector.tensor_tensor(cc, tfc[:, 0:w], tig[:, 0:w], ALU.add)
        th = work.tile([100, 512], BF16, tag="th", bufs=3, name="th")
        nc.scalar.activation(th[:, 0:w], cc, AF.Tanh)
        nc.vector.tensor_tensor(h_dest_fn(c0, c1), A[:, 1024:1024 + w],
                                th[:, 0:w], ALU.mult)


def _emit_cell64(nc, work, psum, parts, nM, c_ap, h_out_ap, first,
                 psum_tag="gg"):
    """One N=64 LSTM cell. Gates in one PSUM tile, layout [i|f|o|g] (nM=4,
    H=100) or [ia ib fa fb oa ob ga gb] (nM=8, H=200).

    parts: list of (lhsT_ap, rhs_ap); each full width; chunk m slices
    lhsT[:, m*100:(m+1)*100]. parts[0] is the primary (start=True).
    """
    W = (nM // 4) * BC  # per-gate total width: 64 or 128
    pg = psum.tile([100, 512], F32, tag=psum_tag,
                   bufs=(4 if psum_tag == "cell" else 2), name="pg64")
    n = len(parts)
    for m in range(nM):
        out_ap = pg[:, m * BC:(m + 1) * BC]
        for j, (lh, rr) in enumerate(parts):
            nc.tensor.matmul(out_ap, lh[:, m * 100:(m + 1) * 100], rr,
                             start=(j == 0), stop=(j == n - 1))
    Aa = work.tile([100, 512], BF16, tag="Ag", bufs=3, name="Aa")
    nsig = 3 * W
    nc.scalar.activation(Aa[:, 0:nsig], pg[:, 0:nsig], AF.Sigmoid)
    nc.scalar.activation(Aa[:, nsig:4 * W], pg[:, nsig:4 * W], AF.Tanh)
    i_s, f_s, o_s, g_s = (Aa[:, k * W:(k + 1) * W] for k in range(4))
    if first:
        nc.vector.tensor_tensor(c_ap, i_s, g_s, ALU.mult)
    else:
        tig = work.tile([100, 128], BF16, tag="tig64", bufs=3, name="tig64")
        nc.vector.tensor_tensor(tig[:, 0:W], i_s, g_s, ALU.mult)
        tfc = work.tile([100, 128], F32, tag="tfc64", bufs=3, name="tfc64")
        nc.vector.tensor_tensor(tfc[:, 0:W], f_s, c_ap, ALU.mult)
        nc.vector.tensor_tensor(c_ap, tfc[:, 0:W], tig[:, 0:W], ALU.add)
    th = work.tile([100, 128], BF16, tag="th64", bufs=3, name="th64")
    nc.scalar.activation(th[:, 0:W], c_ap, AF.Tanh)
    nc.vector.tensor_tensor(h_out_ap, o_s, th[:, 0:W], ALU.mult)


def _emit_shift_phase(nc, work, psum, sp, xt9, yf1, wts, mask_b, s0_out):
    """One shift_accumulate. wts: dict of weight tile APs (dir-major cols).
    Writes the masked 200-feature sum into s0_out (100, 128) bf16 [a|b]."""
    bigY = sp.tile([100 + NV + 1, T * NF + STRIDE], BF16, tag="bigY", bufs=1,
                   name="bigY")
    cL0 = sp.tile([100, NF], F32, tag="cx", bufs=1, name="cL0")
    cF = sp.tile([100, NF], F32, tag="cF", bufs=1, name="cF")
    outB = sp.tile([100, NF], BF16, tag="outB", bufs=1, name="outB")

    def wi0(gi):
        return wts["wi0"][:, 400 + gi * 100: 400 + (gi + 1) * 100]

    def wh0(gi):
        return wts["wh0"][:, 400 + gi * 100: 400 + (gi + 1) * 100]

    def wl1(name, d):
        return lambda gi: wts[name][:, d * 400 + gi * 100: d * 400 + (gi + 1) * 100]

    # --- bwd L0 over the f-batch (writes bigY step-major). For tt>0 the
    # rotated x rows for step tt are DMA'd into rows 100:109 of the previous
    # step's region, so hh+x+bias collapse into one K=109 matmul per gate.
    def wfus(gi):
        return wts["wfus0"][:, 400 + gi * 100: 400 + (gi + 1) * 100]

    for tt in range(T):
        S = (tt + 1) * BC

        def xrhs(c0, c1, S=S, tt=tt):
            out = []
            if c0 < S:
                e = min(c1, S)
                base = (T - 1 - tt) * BC
                out.append((0, e - c0, xt9[:, base + c0: base + e]))
            if c1 > S:
                s = max(c0, S)
                out.append((s - c0, c1 - s, xt9[:, s - S:c1 - S]))
            return out

        if tt > 0:
            pbase = (tt - 1) * NF
            nc.gpsimd.dma_start(bigY[100:100 + NV + 1, pbase:pbase + S],
                                xt9[:, (T - 1 - tt) * BC:(T - 1 - tt) * BC + S])
            if S < NF:
                nc.gpsimd.dma_start(bigY[100:100 + NV + 1, pbase + S:pbase + NF],
                                    xt9[:, 0:NF - S])

            def hxrhs(c0, c1, tt=tt):
                return [(0, c1 - c0,
                         bigY[0:100 + NV + 1,
                              (tt - 1) * NF + c0:(tt - 1) * NF + c1])]
            parts = [(wfus, hxrhs)]
        else:
            parts = [(wi0, xrhs)]
        _emit_macro_step(nc, work, psum, parts, cL0,
                         lambda c0, c1, tt=tt:
                         bigY[0:100, tt * NF + c0:tt * NF + c1],
                         0, NF, first=(tt == 0))

    # --- bwd L1 (shrinking suffix; cB reuses cL0's slot => starts after L0)
    cB = sp.tile([100, NF], F32, tag="cx", bufs=1, name="cB")
    hB_prev = None
    for tt in range(T):
        hB = sp.tile([100, NF], BF16, tag="hB", bufs=2, name="hB")

        def yb_rhs(c0, c1, tt=tt):
            return [(0, c1 - c0, bigY[0:100, tt * NF + c0:tt * NF + c1])]

        def yf_rhs(c0, c1, tt=tt):
            out = []
            S = (tt + 1) * BC
            if c0 < S:
                out.append((0, BC, yf1[:, (T - 1) * BC:T * BC]))
            if c1 > S:
                s = max(c0, S)
                out.append((s - c0, c1 - s, yf1[:, s - S:c1 - S]))
            return out

        parts = [(wl1("wi1b", 1), yb_rhs), (wl1("wi1a", 1), yf_rhs)]
        if tt > 0:
            def hh_rhs(c0, c1, hB_prev=hB_prev):
                return [(0, c1 - c0, hB_prev[:, c0:c1])]
            parts.append((wl1("wh1", 1), hh_rhs))
        _emit_macro_step(nc, work, psum, parts, cB,
                         lambda c0, c1, hB=hB: hB[:, c0:c1],
                         tt * BC, NF, first=(tt == 0))
        nc.vector.tensor_copy(outB[:, tt * BC:(tt + 1) * BC],
                              hB[:, tt * BC:(tt + 1) * BC])
        hB_prev = hB

    # --- fwd L1 (full 24 steps; needs all of bigY)
    hF_prev = None
    outF = None
    for tt in range(T):
        hF = sp.tile([100, NF], BF16, tag="hF", bufs=2, name="hF")

        def yf_rhs(c0, c1, tt=tt):
            nb = (c1 - c0) // BC
            return [(0, c1 - c0,
                     yf1[:, tt * BC:(tt + 1) * BC].unsqueeze(1)
                     .broadcast_to([101, nb, BC]))]

        def yb_rhs(c0, c1, tt=tt):
            out = []
            Sp = (tt + 1) * BC
            if c0 < Sp:
                e = min(c1, Sp)
                f0 = c0 // BC
                nf = (e - c0) // BC
                out.append((0, e - c0,
                            _strided_blocks(bigY[0:100, :],
                                            (T - 1 - tt) * NF + f0 * STRIDE,
                                            nf)))
            if c1 > Sp:
                s = max(c0, Sp)
                f0 = s // BC
                nf = (c1 - s) // BC
                off = f0 * STRIDE - (1 + tt) * NF
                out.append((s - c0, c1 - s,
                            _strided_blocks(bigY[0:100, :], off, nf)))
            return out

        parts = [(wl1("wi1a", 0), yf_rhs), (wl1("wi1b", 0), yb_rhs)]
        if tt > 0:
            def hh_rhs(c0, c1, hF_prev=hF_prev):
                return [(0, c1 - c0, hF_prev[:, c0:c1])]
            parts.append((wl1("wh1", 0), hh_rhs))
        _emit_macro_step(nc, work, psum, parts, cF,
                         lambda c0, c1, hF=hF: hF[:, c0:c1],
                         0, NF, first=(tt == 0))
        hF_prev = hF
        if tt == T - 1:
            outF = hF

    # --- masked accumulation: s0 = sum_f mask[:, f] * [outF; outB]
    for half, src in [(0, outF), (1, outB)]:
        tmp = work.tile([100, NF], F32, tag="redtmp", bufs=1, name="redtmp")
        nc.vector.tensor_tensor(tmp[:], src[:], mask_b[:], ALU.mult)
        red = work.tile([100, BC], F32, tag="red64", bufs=2, name="red64")
        nc.vector.tensor_reduce(
            red[:].unsqueeze(2),
            tmp[:].rearrange("p (f k) -> p k f", f=T),
            mybir.AxisListType.X, ALU.add)
        nc.vector.tensor_copy(s0_out[:, half * BC:(half + 1) * BC], red[:])


def build_kernel():
    nc = bacc.Bacc("TRN2", target_bir_lowering=False, debug=False)
    d = _declare(nc)
    enc_wnames = [f"{p}_{k}" for p in ("e", "s", "c") for k in ENC_WNAMES]
    dec_wnames = (["d_" + k for k in D_WNAMES]
                  + [f"{p}_{k}" for p in ("ds", "dc") for k in DS_WNAMES]
                  + ["lin"])
    with TileContext(nc) as tc:
        with (
            tc.tile_pool(name="const", bufs=1) as const,
            tc.tile_pool(name="persist", bufs=1) as persist,
            tc.tile_pool(name="work", bufs=1) as work,
        ):
            # ---- load encoder-side constants
            wt = {}
            for name in enc_wnames:
                dh = d[name]
                t = const.tile(list(dh.shape), WDT, tag="w_" + name, bufs=1,
                               name="w_" + name)
                nc.sync.dma_start(t[:], dh[:])
                wt[name] = t
            xt9 = const.tile([NV + 1, NF], WDT, tag="xt9t", bufs=1, name="xt9t")
            nc.sync.dma_start(xt9[:], d["xt9"][:])

            def P(shape, dt, name):
                return persist.tile(shape, dt, tag=name, bufs=1, name=name)

            yf1S = P([101, NF], BF16, "yf1S")
            yf1C = P([101, NF], BF16, "yf1C")
            yf1E = P([101, NF], BF16, "yf1E")
            bigYE = P([100, NF], BF16, "bigYE")
            s0_bf = P([100, 128], BF16, "s0_bf")
            sc0_bf = P([100, 128], BF16, "sc0_bf")
            enc_bf = P([100, 128], BF16, "enc_bf")
            maskS_b = P([100, NF], BF16, "maskS_b")
            maskC_b = P([100, NF], BF16, "maskC_b")
            mI = P([100, 128], BF16, "mI")
            mC = P([100, 128], BF16, "mC")
            mBoth = P([1, BC], F32, "mBoth")
            OUT = P([1, 2 * FC * BC], F32, "OUT")
            for yf in (yf1S, yf1C, yf1E):
                # rows 0:100 are fully overwritten by the L0 runs; only the
                # ones-row (bias rhs) must survive
                nc.vector.memset(yf[:], 1.0)

            with tc.tile_pool(name="psum_main", bufs=1, space="PSUM") as psum:
                # ---- masks from xev
                with tc.tile_pool(name="setup", bufs=1) as setup:
                    xev = setup.tile([2, NF], F32, tag="xev_t", bufs=1,
                                     name="xev_t")
                    nc.sync.dma_start(xev[:], d["xev"][:])
                    rows = setup.tile([2, NF], F32, tag="rows", bufs=1,
                                      name="rows")
                    nc.vector.tensor_scalar(rows[:], xev[:], 0.0, None, ALU.is_gt)
                    ones1 = setup.tile([1, 100], F32, tag="ones1", bufs=1,
                                       name="ones1")
                    nc.any.memset(ones1[:], 1.0)
                    # f-batched mask broadcasts (100, NF)
                    for r, dst in [(0, maskS_b), (1, maskC_b)]:
                        for c0 in range(0, NF, 512):
                            pm = psum.tile([100, 512], F32, tag="gg", bufs=2,
                                           name="pmask")
                            nc.tensor.matmul(pm[:], ones1[:],
                                             rows[r:r + 1, c0:c0 + 512],
                                             start=True, stop=True)
                            nc.scalar.copy(dst[:, c0:c0 + 512], pm[:])
                    # per-sample masks (any event over t)
                    sm = setup.tile([2, BC], F32, tag="sm", bufs=1, name="sm")
                    nc.vector.tensor_reduce(
                        sm[:].unsqueeze(2),
                        rows[:].rearrange("p (f k) -> p k f", f=T),
                        mybir.AxisListType.X, ALU.add)
                    smr = setup.tile([2, BC], F32, tag="smr", bufs=1, name="smr")
                    nc.vector.tensor_scalar(smr[:], sm[:], 0.0, None, ALU.is_gt)
                    both = setup.tile([1, BC], F32, tag="both", bufs=1,
                                      name="both")
                    nc.vector.tensor_tensor(both[:], sm[0:1, :], sm[1:2, :],
                                            ALU.add)
                    nc.vector.tensor_scalar(mBoth[:], both[:], 0.0, None,
                                            ALU.is_gt)
                    for r, dst in [(0, mI), (1, mC)]:
                        pm = psum.tile([100, 512], F32, tag="gg", bufs=2,
                                       name="pmask2")
                        nc.tensor.matmul(pm[:, 0:BC], ones1[:], smr[r:r + 1, :],
                                         start=True, stop=True)
                        nc.scalar.copy(dst[:, 0:BC], pm[:, 0:BC])
                        nc.vector.tensor_copy(dst[:, BC:128], dst[:, 0:BC])

                # ---- prologue: fwd-L0 runs for S, C, E (+ E bwd-L0), N=64
                def run_l0_64(wi0_t, wh0_t, d0, rhs_fn, h_dest_fn, ctag):
                    c64 = persist.tile([100, BC], F32, tag=ctag, bufs=1,
                                       name=ctag)
                    h_prev = None
                    for t_ in range(T):
                        parts = [(wi0_t[:, d0 * 400:(d0 + 1) * 400], rhs_fn(t_))]
                        if t_ > 0:
                            parts.append((wh0_t[:, d0 * 400:(d0 + 1) * 400],
                                          h_prev))
                        _emit_cell64(nc, work, psum, parts, 4, c64[:],
                                     h_dest_fn(t_), first=(t_ == 0))
                        h_prev = h_dest_fn(t_)

                run_l0_64(wt["s_wi0"], wt["s_wh0"], 0,
                          lambda t_: xt9[:, t_ * BC:(t_ + 1) * BC],
                          lambda t_: yf1S[0:100, t_ * BC:(t_ + 1) * BC], "c64s")
                run_l0_64(wt["c_wi0"], wt["c_wh0"], 0,
                          lambda t_: xt9[:, t_ * BC:(t_ + 1) * BC],
                          lambda t_: yf1C[0:100, t_ * BC:(t_ + 1) * BC], "c64c")
                run_l0_64(wt["e_wi0"], wt["e_wh0"], 0,
                          lambda t_: xt9[:, t_ * BC:(t_ + 1) * BC],
                          lambda t_: yf1E[0:100, t_ * BC:(t_ + 1) * BC], "c64e")
                run_l0_64(wt["e_wi0"], wt["e_wh0"], 1,
                          lambda t_: xt9[:, (T - 1 - t_) * BC:(T - t_) * BC],
                          lambda t_: bigYE[:, t_ * BC:(t_ + 1) * BC], "c64eb")

                # ---- shift phases (the heavy part)
                s_wts = {k[2:]: v for k, v in wt.items() if k.startswith("s_")}
                c_wts = {k[2:]: v for k, v in wt.items() if k.startswith("c_")}
                with tc.tile_pool(name="shiftS", bufs=1) as sp:
                    _emit_shift_phase(nc, work, psum, sp, xt9, yf1S, s_wts,
                                      maskS_b, s0_bf)
                with tc.tile_pool(name="shiftC", bufs=1) as sp:
                    _emit_shift_phase(nc, work, psum, sp, xt9, yf1C, c_wts,
                                      maskC_b, sc0_bf)

                # ---- encoder fwd L1 + 1-step bwd L1
                cEf1 = P([100, BC], F32, "cEf1")
                hE_prev = None
                for t_ in range(T):
                    hE = work.tile([100, BC], BF16, tag="hEf1", bufs=2,
                                   name="hEf1")
                    parts = [(wt["e_wi1a"][:, 0:400],
                              yf1E[:, t_ * BC:(t_ + 1) * BC]),
                             (wt["e_wi1b"][:, 0:400],
                              bigYE[:, (T - 1 - t_) * BC:(T - t_) * BC])]
                    if t_ > 0:
                        parts.append((wt["e_wh1"][:, 0:400], hE_prev[:]))
                    dest = enc_bf[:, 0:BC] if t_ == T - 1 else hE[:]
                    _emit_cell64(nc, work, psum, parts, 4, cEf1[:], dest,
                                 first=(t_ == 0))
                    hE_prev = hE
                cEb1 = P([100, BC], F32, "cEb1")
                parts = [(wt["e_wi1a"][:, 400:800], yf1E[:, (T - 1) * BC:T * BC]),
                         (wt["e_wi1b"][:, 400:800], bigYE[:, 0:BC])]
                _emit_cell64(nc, work, psum, parts, 4, cEb1[:],
                             enc_bf[:, BC:128], first=True)

            # ---- decoder
            with (
                tc.tile_pool(name="psum_dec", bufs=1, space="PSUM") as psd,
                tc.tile_pool(name="decp", bufs=1) as dp,
            ):
                for name in dec_wnames:
                    dh = d[name]
                    t = dp.tile(list(dh.shape), WDT, tag="w_" + name, bufs=1,
                                name="w_" + name)
                    nc.sync.dma_start(t[:], dh[:])
                    wt[name] = t

                def st_h(name, w):
                    ts = []
                    for k in range(2):
                        t_ = dp.tile([101, w], BF16, tag=f"{name}_{k}", bufs=1,
                                     name=f"{name}_{k}")
                        nc.vector.memset(t_[:], 1.0)
                        nc.vector.memset(t_[0:100, :], 0.0)
                        ts.append(t_)
                    return ts

                def st_c(name, w):
                    return [dp.tile([100, w], F32, tag=f"{name}c_{k}", bufs=1,
                                    name=f"{name}c_{k}") for k in range(2)]

                y_t = st_h("y", 128)
                ssum = dp.tile([101, 128], BF16, tag="ssum", bufs=1, name="ssum")
                nc.vector.memset(ssum[:], 1.0)
                states = {}
                for dec, w in [("d", 64), ("ds", 128), ("dc", 128)]:
                    for lay in ("0", "1"):
                        for dr in ("f", "b"):
                            states[f"{dec}h{lay}{dr}"] = st_h(f"{dec}h{lay}{dr}", w)
                            states[f"{dec}c{lay}{dr}"] = st_c(f"{dec}c{lay}{dr}", w)
                nc.vector.tensor_copy(y_t[1][0:100, :], enc_bf[:])

                def dec_cell(dec, lay, dr, k, xparts, nM, first):
                    cur, prev = k % 2, (k + 1) % 2
                    h_prev = states[f"{dec}h{lay}{dr}"][prev]
                    h_out = states[f"{dec}h{lay}{dr}"][cur]
                    c_t = states[f"{dec}c{lay}{dr}"][0]
                    di = 0 if dr == "f" else 1
                    parts = list(xparts(di))
                    if not first:
                        if dec == "d":
                            parts.append((wt[f"d_wh{lay}"][:, di * 400:(di + 1) * 400],
                                          h_prev[0:100, :]))
                        else:
                            parts.append((wt[f"{dec}_wh{lay}a"][:, di * 800:(di + 1) * 800],
                                          h_prev[0:100, 0:BC]))
                            parts.append((wt[f"{dec}_wh{lay}b"][:, di * 800:(di + 1) * 800],
                                          h_prev[0:100, BC:128]))
                    _emit_cell64(nc, work, psd, parts, nM, c_t[:],
                                 h_out[0:100, :], first, psum_tag="cell")
                    return h_out

                for k in range(FC):
                    cur, prev = k % 2, (k + 1) % 2
                    y_prev = y_t[prev]
                    first = (k == 0)
                    if k == 0:
                        s_a, s_b = s0_bf[:, 0:BC], s0_bf[:, BC:128]
                        sc_a, sc_b = sc0_bf[:, 0:BC], sc0_bf[:, BC:128]
                    else:
                        s_a = states["dsh1b"][prev][0:100, 0:BC]
                        s_b = states["dsh1b"][prev][0:100, BC:128]
                        sc_a = states["dch1b"][prev][0:100, 0:BC]
                        sc_b = states["dch1b"][prev][0:100, BC:128]

                    def d_x0(di, y_prev=y_prev):
                        return [(wt["d_wi0a"][:, di * 400:(di + 1) * 400],
                                 y_prev[0:101, 0:BC]),
                                (wt["d_wi0b"][:, di * 400:(di + 1) * 400],
                                 y_prev[0:100, BC:128])]
                    h0f = dec_cell("d", "0", "f", k, d_x0, 4, first)
                    h0b = dec_cell("d", "0", "b", k, d_x0, 4, first)

                    def d_x1(di, h0f=h0f, h0b=h0b):
                        return [(wt["d_wi1a"][:, di * 400:(di + 1) * 400],
                                 h0f[0:101, :]),
                                (wt["d_wi1b"][:, di * 400:(di + 1) * 400],
                                 h0b[0:100, :])]
                    yd_a = dec_cell("d", "1", "f", k, d_x1, 4, first)
                    yd_b = dec_cell("d", "1", "b", k, d_x1, 4, first)

                    outs = {}
                    for dec, (sa, sb) in [("ds", (s_a, s_b)),
                                          ("dc", (sc_a, sc_b))]:
                        def s_x0(di, dec=dec, sa=sa, sb=sb, y_prev=y_prev):
                            return [
                                (wt[f"{dec}_wi0a"][:, di * 800:(di + 1) * 800],
                                 y_prev[0:101, 0:BC]),
                                (wt[f"{dec}_wi0b"][:, di * 800:(di + 1) * 800],
                                 y_prev[0:100, BC:128]),
                                (wt[f"{dec}_wi0c"][:, di * 800:(di + 1) * 800], sa),
                                (wt[f"{dec}_wi0d"][:, di * 800:(di + 1) * 800], sb),
                            ]
                        g0f = dec_cell(dec, "0", "f", k, s_x0, 8, first)
                        g0b = dec_cell(dec, "0", "b", k, s_x0, 8, first)

                        def s_x1(di, dec=dec, g0f=g0f, g0b=g0b):
                            return [
                                (wt[f"{dec}_wi1a"][:, di * 800:(di + 1) * 800],
                                 g0f[0:101, 0:BC]),
                                (wt[f"{dec}_wi1b"][:, di * 800:(di + 1) * 800],
                                 g0f[0:100, BC:128]),
                                (wt[f"{dec}_wi1c"][:, di * 800:(di + 1) * 800],
                                 g0b[0:100, 0:BC]),
                                (wt[f"{dec}_wi1d"][:, di * 800:(di + 1) * 800],
                                 g0b[0:100, BC:128]),
                            ]
                        outs[dec + "f"] = dec_cell(dec, "1", "f", k, s_x1, 8,
                                                   first)
                        outs[dec + "b"] = dec_cell(dec, "1", "b", k, s_x1, 8,
                                                   first)

                    # --- yx = yd - mI*relu(ySf) + mC*relu(ySCf)
                    y_new = y_t[cur]
                    r1 = work.tile([100, 128], BF16, tag="r1", bufs=2, name="r1")
                    nc.vector.tensor_scalar(r1[:], outs["dsf"][0:100, :], 0.0,
                                            None, ALU.max)
                    rm1 = work.tile([100, 128], BF16, tag="rm1", bufs=2,
                                    name="rm1")
                    nc.vector.tensor_tensor(rm1[:], r1[:], mI[:], ALU.mult)
                    nc.vector.tensor_tensor(y_new[0:100, 0:BC], yd_a[0:100, :],
                                            rm1[:, 0:BC], ALU.subtract)
                    nc.vector.tensor_tensor(y_new[0:100, BC:128], yd_b[0:100, :],
                                            rm1[:, BC:128], ALU.subtract)
                    r2 = work.tile([100, 128], BF16, tag="r1", bufs=2, name="r2")
                    nc.vector.tensor_scalar(r2[:], outs["dcf"][0:100, :], 0.0,
                                            None, ALU.max)
                    rm2 = work.tile([100, 128], BF16, tag="rm1", bufs=2,
                                    name="rm2")
                    nc.vector.tensor_tensor(rm2[:], r2[:], mC[:], ALU.mult)
                    nc.vector.tensor_tensor(y_new[0:100, 0:BC],
                                            y_new[0:100, 0:BC],
                                            rm2[:, 0:BC], ALU.add)
                    nc.vector.tensor_tensor(y_new[0:100, BC:128],
                                            y_new[0:100, BC:128],
                                            rm2[:, BC:128], ALU.add)

                    # --- out_f = yx @ lin_w.T + lin_b
                    pl = psd.tile([1, BC], F32, tag="lin", bufs=2, name="pl")
                    nc.tensor.matmul(pl[:], wt["lin"][:, 0:1],
                                     y_new[0:101, 0:BC], start=True, stop=False)
                    nc.tensor.matmul(pl[:], wt["lin"][0:100, 1:2],
                                     y_new[0:100, BC:128], start=False, stop=True)
                    nc.scalar.copy(OUT[:, k * BC:(k + 1) * BC], pl[:])

                    # --- outS = mBoth * ((mI*ySb + mC*ySCb) @ linS_w.T + linS_b)
                    t1 = work.tile([100, 128], BF16, tag="r1", bufs=2, name="t1")
                    nc.vector.tensor_tensor(t1[:], outs["dsb"][0:100, :], mI[:],
                                            ALU.mult)
                    t2 = work.tile([100, 128], BF16, tag="rm1", bufs=2, name="t2")
                    nc.vector.tensor_tensor(t2[:], outs["dcb"][0:100, :], mC[:],
                                            ALU.mult)
                    nc.vector.tensor_tensor(ssum[0:100, :], t1[:], t2[:], ALU.add)
                    pl2 = psd.tile([1, BC], F32, tag="lin", bufs=2, name="pl2")
                    nc.tensor.matmul(pl2[:], wt["lin"][:, 2:3],
                                     ssum[0:101, 0:BC], start=True, stop=False)
                    nc.tensor.matmul(pl2[:], wt["lin"][0:100, 3:4],
                                     ssum[0:100, BC:128], start=False, stop=True)
                    rowS = work.tile([1, BC], F32, tag="rowS", bufs=2,
                                     name="rowS")
                    nc.scalar.copy(rowS[:], pl2[:])
                    nc.vector.tensor_tensor(OUT[:, (FC + k) * BC:(FC + k + 1) * BC],
                                            rowS[:], mBoth[:], ALU.mult)

                nc.sync.dma_start(d["out"][:], OUT[:])
    return nc


_CACHED = None


def _install_ntff_shim():
    """Register a minimal antenv.axon_hooks so trace=True works under axon
    (only used when KERNEL_TRACE=1; the plain run never needs it)."""
    import contextlib
    import ctypes
    import types
    import glob

    hook = None
    cands = glob.glob("/opt/axon/libaxon_pjrt.so") + glob.glob(
        "/root/.axon_site/**/libaxon_pjrt.so", recursive=True)
    for so_path in cands:
        try:
            lib = ctypes.CDLL(so_path)
        except OSError:
            continue
        if not hasattr(lib, "axon_start_nrt_profile"):
            continue
        lib.axon_start_nrt_profile.argtypes = [
            ctypes.POINTER(ctypes.c_int64), ctypes.c_size_t]
        lib.axon_start_nrt_profile.restype = ctypes.c_int64
        lib.axon_stop_nrt_profile.argtypes = [ctypes.c_char_p]
        lib.axon_stop_nrt_profile.restype = ctypes.c_int64

        @contextlib.contextmanager
        def _hook(output_dir, device_ids, lib=lib):
            import jax
            jax.devices()
            if device_ids:
                ids = (ctypes.c_int64 * len(device_ids))(*device_ids)
                rc = lib.axon_start_nrt_profile(ids, len(device_ids))
            else:
                rc = lib.axon_start_nrt_profile(None, 0)
            if rc != 0:
                raise RuntimeError(f"axon_start_nrt_profile rc={rc}")
            try:
                yield
            finally:
                n = lib.axon_stop_nrt_profile(str(output_dir).encode())
                print(f"ntff profile: {n} file(s) -> {output_dir}",
                      file=sys.stderr)
        hook = _hook
        break
    mod = types.ModuleType("antenv.axon_hooks")
    mod.get_axon_ntff_profile_hook = lambda: hook
    mod.set_axon_ntff_profile_hook = lambda h: None
    import antenv
    antenv.axon_hooks = mod
    sys.modules["antenv.axon_hooks"] = mod


def kernel(**inputs):
    global _CACHED
    if _CACHED is None:
        _CACHED = build_kernel()
    nc = _CACHED
    in_maps = _prep_inputs(**inputs)
    trace = os.environ.get("KERNEL_TRACE", "0") == "1"
    if trace:
        _install_ntff_shim()
        r = run_bass_kernel_spmd(nc, in_maps, list(range(NCORES)), trace=True)
        print(f"HW exec time: {r.exec_time_ns} ns")
        res = r.results
    else:
        res = run_bass_kernel_spmd(nc, in_maps, list(range(NCORES))).results
    outer = np.zeros((BP, FC), np.float32)
    outS = np.zeros((BP, FC), np.float32)
    for c in range(NCORES):
        o = res[c]["out"]  # (24, 64)
        outer[c * BC:(c + 1) * BC] = o[0:FC].T
        outS[c * BC:(c + 1) * BC] = o[FC:2 * FC].T
    return outer[:B], outS[:B]


# revision 14
# speedup vs baseline: 1.1542x; 1.0095x over previous
"""Trainium2 Bass kernel for nn_Block_66073776882206 (ragged_sequence).

Strategy
--------
Pure data parallelism over the batch: pad 500 -> 512, shard 64 samples per
NeuronCore across 8 cores, replicate all weights. Everything on-device is
feature-major: SBUF tiles are (features<=128, batch-cols), LSTM cell matmuls
use lhsT = weight chunks (K=feat_in, M=gate_rows), rhs = activations
(K, N=batch-cols), PSUM out (gate_rows, batch-cols).

The heavy part (shift_accumulate) runs, for each backcast step f in [0,24),
a two-segment bidirectional 2-layer LSTM over the length-24 sequence. The
f-axis is batched into the matmul N dimension: N = 24 f-blocks x 64 batch =
1536 columns. The permutation perm_f[t] = (f-1-t if t<f else 23-(t-f)) is
linear in f with slope 1 in both branches, so every permuted read/write
reduces to a contiguous slice or a stride-1600 access pattern on a
step-major store -- no gather DMAs. The backward L1 run for block f only
needs steps 0..f (its consumed output is the processing-step-f entry), so
it runs on a shrinking column suffix (~52% of the full work).

Gate order is host-reordered to [i, f, o, g] so one ACT instruction applies
sigmoid across i,f,o and one applies tanh to g. Biases are folded into the
x-side matmul via an appended ones-row on the rhs / bias-row on the lhsT.

Matmul inputs are bf16; PSUM accumulation and the cell state c stay fp32.
"""
import sys
import os

sys.path.insert(0, "/opt/trn_rl_repo")

import numpy as np
import ml_dtypes

import concourse.bass as bass
import concourse.bacc as bacc
import concourse.mybir as mybir
from concourse.tile import TileContext
from concourse.bass_utils import run_bass_kernel_spmd

F32 = mybir.dt.float32
BF16 = mybir.dt.bfloat16
AF = mybir.ActivationFunctionType
ALU = mybir.AluOpType

NV = 8
H = 100          # UNITS
T = 24           # BACKLEN
FC = 12          # FORECAST
B = 500
NCORES = 8
BC = 64          # batch per core (padded)
BP = NCORES * BC  # 512
NF = T * BC      # 1536 f-batched columns
STRIDE = NF + BC  # 1600: f-block stride between (step, block) diagonals
H2 = 2 * H

WDT = BF16
NP_WDT = ml_dtypes.bfloat16

GATE_PERM = [0, 1, 3, 2]  # reference order [i, f, g, o] -> ours [i, f, o, g]

ENC_WNAMES = ["wi0", "wh0", "wfus0", "wi1a", "wi1b", "wh1"]
D_WNAMES = ["wi0a", "wi0b", "wh0", "wi1a", "wi1b", "wh1"]
DS_WNAMES = ["wi0a", "wi0b", "wi0c", "wi0d", "wh0a", "wh0b",
             "wi1a", "wi1b", "wi1c", "wi1d", "wh1a", "wh1b"]


# ----------------------------------------------------------------------------
# host-side weight prep
# ----------------------------------------------------------------------------

def _reorder(M):
    """(4H, X) -> rows gate-reordered to [i, f, o, g]."""
    Hq = M.shape[0] // 4
    return M.reshape(4, Hq, -1)[GATE_PERM].reshape(4 * Hq, -1)


def _wi_aug(Wi, b, zero_ch=()):
    """W_ih (4H, I), b (4H,) -> lhsT (I+1, 4H): rows = input features + a
    bias row (applied via the ones-row in rhs); cols = reordered gate rows."""
    W = np.array(Wi, dtype=np.float32).copy()
    for c in zero_ch:
        W[:, c] = 0.0
    W = _reorder(W)
    bb = _reorder(np.asarray(b, np.float32).reshape(-1, 1))
    return np.concatenate([W.T, bb.T], axis=0)


def _wh(Wh):
    return np.ascontiguousarray(_reorder(np.asarray(Wh, np.float32)).T)


def _enc_pack(p, zero_ch):
    """2-layer bidir LSTM with H=100, I=8 -> dict of (K, 2*400) arrays."""
    out = {k: [] for k in ENC_WNAMES}
    for d in range(2):
        full0 = _wi_aug(p["W_ih0"][d], p["b0"][d], zero_ch)      # (9, 400)
        out["wi0"].append(full0)
        wh0 = _wh(p["W_hh0"][d])                                  # (100, 400)
        out["wh0"].append(wh0)
        out["wfus0"].append(np.concatenate([wh0, full0], 0))      # (109, 400)
        full1 = _wi_aug(p["W_ih1"][d], p["b1"][d])                # (201, 400)
        out["wi1a"].append(np.concatenate([full1[0:H], full1[2 * H:2 * H + 1]], 0))
        out["wi1b"].append(full1[H:2 * H])
        out["wh1"].append(_wh(p["W_hh1"][d]))
    return {k: np.concatenate(v, 1) for k, v in out.items()}


def _dec_pack(p):
    """p_dec: H=100, I=200."""
    out = {k: [] for k in D_WNAMES}
    for d in range(2):
        full0 = _wi_aug(p["W_ih0"][d], p["b0"][d])                # (201, 400)
        out["wi0a"].append(np.concatenate([full0[0:H], full0[2 * H:2 * H + 1]], 0))
        out["wi0b"].append(full0[H:2 * H])
        out["wh0"].append(_wh(p["W_hh0"][d]))
        full1 = _wi_aug(p["W_ih1"][d], p["b1"][d])
        out["wi1a"].append(np.concatenate([full1[0:H], full1[2 * H:2 * H + 1]], 0))
        out["wi1b"].append(full1[H:2 * H])
        out["wh1"].append(_wh(p["W_hh1"][d]))
    return {k: np.concatenate(v, 1) for k, v in out.items()}


def _decS_pack(p):
    """p_decS / p_decSC: H=200, I=400, 4H=800. K-chunks of 100 rows."""
    out = {k: [] for k in DS_WNAMES}
    for d in range(2):
        full0 = _wi_aug(p["W_ih0"][d], p["b0"][d])                # (401, 800)
        out["wi0a"].append(np.concatenate([full0[0:100], full0[400:401]], 0))
        out["wi0b"].append(full0[100:200])
        out["wi0c"].append(full0[200:300])
        out["wi0d"].append(full0[300:400])
        w0 = _wh(p["W_hh0"][d])                                    # (200, 800)
        out["wh0a"].append(w0[0:100])
        out["wh0b"].append(w0[100:200])
        full1 = _wi_aug(p["W_ih1"][d], p["b1"][d])
        out["wi1a"].append(np.concatenate([full1[0:100], full1[400:401]], 0))
        out["wi1b"].append(full1[100:200])
        out["wi1c"].append(full1[200:300])
        out["wi1d"].append(full1[300:400])
        w1 = _wh(p["W_hh1"][d])
        out["wh1a"].append(w1[0:100])
        out["wh1b"].append(w1[100:200])
    return {k: np.concatenate(v, 1) for k, v in out.items()}  # (100/101, 1600)


def _prep_inputs(xt, xorig, p_lstm, p_dec, p_lstmS, p_decS, p_lstmSC, p_decSC,
                 lin_w, lin_b, linS_w, linS_b):
    """Build the per-core input maps. Weights replicated; x sharded."""
    xt = np.asarray(xt, np.float32)
    xorig = np.asarray(xorig, np.float32)
    xt_p = np.zeros((BP, T, NV), np.float32)
    xo_p = np.zeros((BP, T, NV), np.float32)
    xt_p[:B] = xt
    xo_p[:B] = xorig

    weights = {}
    for pref, pk, zc in [("e", p_lstm, (1, 2)), ("s", p_lstmS, (2,)),
                         ("c", p_lstmSC, (1,))]:
        for k, v in _enc_pack(pk, zc).items():
            weights[pref + "_" + k] = v.astype(NP_WDT)
    for k, v in _dec_pack(p_dec).items():
        weights["d_" + k] = v.astype(NP_WDT)
    for k, v in _decS_pack(p_decS).items():
        weights["ds_" + k] = v.astype(NP_WDT)
    for k, v in _decS_pack(p_decSC).items():
        weights["dc_" + k] = v.astype(NP_WDT)

    lin = np.zeros((101, 4), np.float32)
    lin_w = np.asarray(lin_w, np.float32)
    linS_w = np.asarray(linS_w, np.float32)
    lin[0:100, 0] = lin_w[0, 0:100]
    lin[100, 0] = np.asarray(lin_b, np.float32).reshape(-1)[0]
    lin[0:100, 1] = lin_w[0, 100:200]
    lin[0:100, 2] = linS_w[0, 0:100]
    lin[100, 2] = np.asarray(linS_b, np.float32).reshape(-1)[0]
    lin[0:100, 3] = linS_w[0, 100:200]
    weights["lin"] = lin.astype(NP_WDT)

    in_maps = []
    for c in range(NCORES):
        xs = xt_p[c * BC:(c + 1) * BC]          # (64, 24, 8)
        xo = xo_p[c * BC:(c + 1) * BC]
        xt9 = np.ones((NV + 1, NF), np.float32)
        xt9[0:NV] = xs.transpose(2, 1, 0).reshape(NV, NF)   # [ch, t*64+b]
        xev = xo.transpose(2, 1, 0)[1:3].reshape(2, NF)     # [ch-1, t*64+b]
        m = dict(weights)
        m["eye64"] = np.eye(BC, dtype=np.float32).astype(NP_WDT)
        m["xt9"] = xt9.astype(NP_WDT)
        m["xev"] = np.ascontiguousarray(xev.astype(np.float32))
        in_maps.append(m)
    return in_maps


# ----------------------------------------------------------------------------
# device kernel
# ----------------------------------------------------------------------------

def _declare(nc):
    d = {}
    d["xt9"] = nc.declare_dram_parameter("xt9", [NV + 1, NF], WDT, isOutput=False)
    d["xev"] = nc.declare_dram_parameter("xev", [2, NF], F32, isOutput=False)
    shapes = {}
    for pref in ["e", "s", "c"]:
        shapes[pref + "_wi0"] = [NV + 1, 800]
        shapes[pref + "_wh0"] = [100, 800]
        shapes[pref + "_wfus0"] = [100 + NV + 1, 800]
        shapes[pref + "_wi1a"] = [101, 800]
        shapes[pref + "_wi1b"] = [100, 800]
        shapes[pref + "_wh1"] = [100, 800]
    for k in D_WNAMES:
        shapes["d_" + k] = [101 if k in ("wi0a", "wi1a") else 100, 800]
    for pref in ["ds", "dc"]:
        for k in DS_WNAMES:
            shapes[pref + "_" + k] = [101 if k in ("wi0a", "wi1a") else 100, 1600]
    shapes["lin"] = [101, 4]
    shapes["eye64"] = [BC, BC]
    for k, shp in shapes.items():
        d[k] = nc.declare_dram_parameter(k, shp, WDT, isOutput=False)
    d["out"] = nc.declare_dram_parameter("out", [2 * FC, BC], F32, isOutput=True)
    return d


def _strided_blocks(big, off, nf):
    """AP over nf 64-wide blocks spaced STRIDE apart starting at col off."""
    return big[:, off:off + nf * STRIDE].rearrange(
        "p (f k) -> p f k", k=STRIDE)[:, :, 0:BC]


def _emit_macro_step(nc, work, psum, parts, c_tile, h_dest_fn, n_lo, n_hi, first):
    """One f-batched LSTM step over active cols [n_lo, n_hi).

    parts: list of (lhsT_fn, rhs_fn). lhsT_fn(gi) -> AP (K, 100).
    rhs_fn(c0, c1) -> list of (off, width, rhs_ap) covering [c0,c1) disjointly
    (off relative to c0). parts[0] must be a single full-width piece when
    len(parts) > 1. h_dest_fn(c0, c1) -> AP for the final h write.
    """
    single = len(parts) == 1
    for c0 in range(n_lo, n_hi, 512):
        c1 = min(c0 + 512, n_hi)
        w = c1 - c0
        pifo = psum.tile([100, 1536], F32, tag="ifo", bufs=2, name="pifo")
        pgg = psum.tile([100, 512], F32, tag="gg", bufs=2, name="pgg")
        for gi in range(4):
            out_ap = pifo[:, gi * 512: gi * 512 + w] if gi < 3 else pgg[:, 0:w]
            mms = []
            for pi, (lhsT_fn, rhs_fn) in enumerate(parts):
                for (off, ww, rhs_ap) in rhs_fn(c0, c1):
                    mms.append((lhsT_fn(gi), off, ww, rhs_ap, pi == 0))
            n = len(mms)
            for j, (lh, off, ww, rr, is_primary) in enumerate(mms):
                # column-disjoint pieces with no accumulation on top are each
                # their own start+stop group
                nc.tensor.matmul(out_ap[:, off:off + ww], lh, rr,
                                 start=(True if single else j == 0),
                                 stop=(True if single else j == n - 1))
        A = work.tile([100, 1536], BF16, tag="A", bufs=3, name="A")
        Ag = work.tile([100, 512], BF16, tag="Ag", bufs=3, name="Ag")
        if w == 512:
            nc.scalar.activation(A[:, 0:1536], pifo[:, 0:1536], AF.Sigmoid)
        else:
            src = pifo[:, 0:1536].rearrange("p (g k) -> p g k", k=512)[:, :, 0:w]
            dst = A[:, 0:1536].rearrange("p (g k) -> p g k", k=512)[:, :, 0:w]
            nc.scalar.activation(dst, src, AF.Sigmoid)
        nc.scalar.activation(Ag[:, 0:w], pgg[:, 0:w], AF.Tanh)
        cc = c_tile[:, c0:c1]
        if first:
            nc.vector.tensor_tensor(cc, A[:, 0:w], Ag[:, 0:w], ALU.mult)
        else:
            tig = work.tile([100, 512], BF16, tag="tig", bufs=3, name="tig")
            nc.vector.tensor_tensor(tig[:, 0:w], A[:, 0:w], Ag[:, 0:w], ALU.mult)
            tfc = work.tile([100, 512], F32, tag="tfc", bufs=2, name="tfc")
            nc.vector.tensor_tensor(tfc[:, 0:w], A[:, 512:512 + w], cc, ALU.mult)
            nc.vector.tensor_tensor(cc, tfc[:, 0:w], tig[:, 0:w], ALU.add)
        th = work.tile([100, 512], BF16, tag="th", bufs=3, name="th")
        nc.scalar.activation(th[:, 0:w], cc, AF.Tanh)
        nc.vector.tensor_tensor(h_dest_fn(c0, c1), A[:, 1024:1024 + w],
                                th[:, 0:w], ALU.mult)


def _emit_cell64(nc, work, psum, parts, nM, c_ap, h_out_ap, first,
                 psum_tag="gg"):
    """One N=64 LSTM cell. Gates in one PSUM tile, layout [i|f|o|g] (nM=4,
    H=100) or [ia ib fa fb oa ob ga gb] (nM=8, H=200).

    parts: list of (lhsT_ap, rhs_ap); each full width; chunk m slices
    lhsT[:, m*100:(m+1)*100]. parts[0] is the primary (start=True).
    """
    W = (nM // 4) * BC  # per-gate total width: 64 or 128
    pg = psum.tile([100, 512], F32, tag=psum_tag,
                   bufs=(4 if psum_tag == "cell" else 2), name="pg64")
    n = len(parts)
    for m in range(nM):
        out_ap = pg[:, m * BC:(m + 1) * BC]
        for j, (lh, rr) in enumerate(parts):
            nc.tensor.matmul(out_ap, lh[:, m * 100:(m + 1) * 100], rr,
                             start=(j == 0), stop=(j == n - 1))
    Aa = work.tile([100, 512], BF16, tag="Ag", bufs=3, name="Aa")
    nsig = 3 * W
    nc.scalar.activation(Aa[:, 0:nsig], pg[:, 0:nsig], AF.Sigmoid)
    nc.scalar.activation(Aa[:, nsig:4 * W], pg[:, nsig:4 * W], AF.Tanh)
    i_s, f_s, o_s, g_s = (Aa[:, k * W:(k + 1) * W] for k in range(4))
    if first:
        nc.vector.tensor_tensor(c_ap, i_s, g_s, ALU.mult)
    else:
        tig = work.tile([100, 128], BF16, tag="tig64", bufs=3, name="tig64")
        nc.vector.tensor_tensor(tig[:, 0:W], i_s, g_s, ALU.mult)
        tfc = work.tile([100, 128], F32, tag="tfc64", bufs=3, name="tfc64")
        nc.vector.tensor_tensor(tfc[:, 0:W], f_s, c_ap, ALU.mult)
        nc.vector.tensor_tensor(c_ap, tfc[:, 0:W], tig[:, 0:W], ALU.add)
    th = work.tile([100, 128], BF16, tag="th64", bufs=3, name="th64")
    nc.scalar.activation(th[:, 0:W], c_ap, AF.Tanh)
    nc.vector.tensor_tensor(h_out_ap, o_s, th[:, 0:W], ALU.mult)


def _emit_shift_phase(nc, work, psum, sp, xt9, yf1, wts, mask_b, s0_out):
    """One shift_accumulate. wts: dict of weight tile APs (dir-major cols).
    Writes the masked 200-feature sum into s0_out (100, 128) bf16 [a|b]."""
    bigY = sp.tile([100 + NV + 1, T * NF + STRIDE], BF16, tag="bigY", bufs=1,
                   name="bigY")
    cL0 = sp.tile([100, NF], F32, tag="cx", bufs=1, name="cL0")
    cF = sp.tile([100, NF], F32, tag="cF", bufs=1, name="cF")
    outB = sp.tile([100, NF], BF16, tag="outB", bufs=1, name="outB")

    def wi0(gi):
        return wts["wi0"][:, 400 + gi * 100: 400 + (gi + 1) * 100]

    def wh0(gi):
        return wts["wh0"][:, 400 + gi * 100: 400 + (gi + 1) * 100]

    def wl1(name, d):
        return lambda gi: wts[name][:, d * 400 + gi * 100: d * 400 + (gi + 1) * 100]

    # --- bwd L0 over the f-batch (writes bigY step-major). For tt>0 the
    # rotated x rows for step tt are DMA'd into rows 100:109 of the previous
    # step's region, so hh+x+bias collapse into one K=109 matmul per gate.
    def wfus(gi):
        return wts["wfus0"][:, 400 + gi * 100: 400 + (gi + 1) * 100]

    for tt in range(T):
        S = (tt + 1) * BC

        def xrhs(c0, c1, S=S, tt=tt):
            out = []
            if c0 < S:
                e = min(c1, S)
                base = (T - 1 - tt) * BC
                out.append((0, e - c0, xt9[:, base + c0: base + e]))
            if c1 > S:
                s = max(c0, S)
                out.append((s - c0, c1 - s, xt9[:, s - S:c1 - S]))
            return out

        if tt > 0:
            pbase = (tt - 1) * NF
            nc.gpsimd.dma_start(bigY[100:100 + NV + 1, pbase:pbase + S],
                                xt9[:, (T - 1 - tt) * BC:(T - 1 - tt) * BC + S])
            if S < NF:
                nc.gpsimd.dma_start(bigY[100:100 + NV + 1, pbase + S:pbase + NF],
                                    xt9[:, 0:NF - S])

            def hxrhs(c0, c1, tt=tt):
                return [(0, c1 - c0,
                         bigY[0:100 + NV + 1,
                              (tt - 1) * NF + c0:(tt - 1) * NF + c1])]
            parts = [(wfus, hxrhs)]
        else:
            parts = [(wi0, xrhs)]
        _emit_macro_step(nc, work, psum, parts, cL0,
                         lambda c0, c1, tt=tt:
                         bigY[0:100, tt * NF + c0:tt * NF + c1],
                         0, NF, first=(tt == 0))

    # --- bwd L1 (shrinking suffix; cB reuses cL0's slot => starts after L0)
    cB = sp.tile([100, NF], F32, tag="cx", bufs=1, name="cB")
    hB_prev = None
    for tt in range(T):
        hB = sp.tile([100, NF], BF16, tag="hB", bufs=2, name="hB")

        def yb_rhs(c0, c1, tt=tt):
            return [(0, c1 - c0, bigY[0:100, tt * NF + c0:tt * NF + c1])]

        def yf_rhs(c0, c1, tt=tt):
            out = []
            S = (tt + 1) * BC
            if c0 < S:
                out.append((0, BC, yf1[:, (T - 1) * BC:T * BC]))
            if c1 > S:
                s = max(c0, S)
                out.append((s - c0, c1 - s, yf1[:, s - S:c1 - S]))
            return out

        parts = [(wl1("wi1b", 1), yb_rhs), (wl1("wi1a", 1), yf_rhs)]
        if tt > 0:
            def hh_rhs(c0, c1, hB_prev=hB_prev):
                return [(0, c1 - c0, hB_prev[:, c0:c1])]
            parts.append((wl1("wh1", 1), hh_rhs))
        _emit_macro_step(nc, work, psum, parts, cB,
                         lambda c0, c1, hB=hB: hB[:, c0:c1],
                         tt * BC, NF, first=(tt == 0))
        nc.vector.tensor_copy(outB[:, tt * BC:(tt + 1) * BC],
                              hB[:, tt * BC:(tt + 1) * BC])
        hB_prev = hB

    # --- fwd L1 (full 24 steps; needs all of bigY)
    hF_prev = None
    outF = None
    for tt in range(T):
        hF = sp.tile([100, NF], BF16, tag="hF", bufs=2, name="hF")

        def yf_rhs(c0, c1, tt=tt):
            nb = (c1 - c0) // BC
            return [(0, c1 - c0,
                     yf1[:, tt * BC:(tt + 1) * BC].unsqueeze(1)
                     .broadcast_to([101, nb, BC]))]

        def yb_rhs(c0, c1, tt=tt):
            out = []
            Sp = (tt + 1) * BC
            if c0 < Sp:
                e = min(c1, Sp)
                f0 = c0 // BC
                nf = (e - c0) // BC
                out.append((0, e - c0,
                            _strided_blocks(bigY[0:100, :],
                                            (T - 1 - tt) * NF + f0 * STRIDE,
                                            nf)))
            if c1 > Sp:
                s = max(c0, Sp)
                f0 = s // BC
                nf = (c1 - s) // BC
                off = f0 * STRIDE - (1 + tt) * NF
                out.append((s - c0, c1 - s,
                            _strided_blocks(bigY[0:100, :], off, nf)))
            return out

        parts = [(wl1("wi1a", 0), yf_rhs), (wl1("wi1b", 0), yb_rhs)]
        if tt > 0:
            def hh_rhs(c0, c1, hF_prev=hF_prev):
                return [(0, c1 - c0, hF_prev[:, c0:c1])]
            parts.append((wl1("wh1", 0), hh_rhs))
        _emit_macro_step(nc, work, psum, parts, cF,
                         lambda c0, c1, hF=hF: hF[:, c0:c1],
                         0, NF, first=(tt == 0))
        hF_prev = hF
        if tt == T - 1:
            outF = hF

    # --- masked accumulation: s0 = sum_f mask[:, f] * [outF; outB]
    for half, src in [(0, outF), (1, outB)]:
        tmp = work.tile([100, NF], F32, tag="redtmp", bufs=1, name="redtmp")
        nc.vector.tensor_tensor(tmp[:], src[:], mask_b[:], ALU.mult)
        red = work.tile([100, BC], F32, tag="red64", bufs=2, name="red64")
        nc.vector.tensor_reduce(
            red[:].unsqueeze(2),
            tmp[:].rearrange("p (f k) -> p k f", f=T),
            mybir.AxisListType.X, ALU.add)
        nc.vector.tensor_copy(s0_out[:, half * BC:(half + 1) * BC], red[:])


def build_kernel():
    nc = bacc.Bacc("TRN2", target_bir_lowering=False, debug=False)
    d = _declare(nc)
    enc_wnames = [f"{p}_{k}" for p in ("e", "s", "c") for k in ENC_WNAMES]
    dec_wnames = (["d_" + k for k in D_WNAMES]
                  + [f"{p}_{k}" for p in ("ds", "dc") for k in DS_WNAMES]
                  + ["lin", "eye64"])
    with TileContext(nc) as tc:
        with (
            tc.tile_pool(name="const", bufs=1) as const,
            tc.tile_pool(name="persist", bufs=1) as persist,
            tc.tile_pool(name="work", bufs=1) as work,
        ):
            # ---- load encoder-side constants
            wt = {}
            for name in enc_wnames:
                dh = d[name]
                t = const.tile(list(dh.shape), WDT, tag="w_" + name, bufs=1,
                               name="w_" + name)
                nc.sync.dma_start(t[:], dh[:])
                wt[name] = t
            xt9 = const.tile([NV + 1, NF], WDT, tag="xt9t", bufs=1, name="xt9t")
            nc.sync.dma_start(xt9[:], d["xt9"][:])

            def P(shape, dt, name):
                return persist.tile(shape, dt, tag=name, bufs=1, name=name)

            yf1S = P([101, NF], BF16, "yf1S")
            yf1C = P([101, NF], BF16, "yf1C")
            yf1E = P([101, NF], BF16, "yf1E")
            bigYE = P([100, NF], BF16, "bigYE")
            s0_bf = P([100, 128], BF16, "s0_bf")
            sc0_bf = P([100, 128], BF16, "sc0_bf")
            enc_bf = P([100, 128], BF16, "enc_bf")
            maskS_b = P([100, NF], BF16, "maskS_b")
            maskC_b = P([100, NF], BF16, "maskC_b")
            mI = P([100, 128], BF16, "mI")
            mC = P([100, 128], BF16, "mC")
            mBoth = P([1, BC], F32, "mBoth")
            OUT = P([1, 2 * FC * BC], F32, "OUT")
            for yf in (yf1S, yf1C, yf1E):
                # rows 0:100 are fully overwritten by the L0 runs; only the
                # ones-row (bias rhs) must survive
                nc.vector.memset(yf[:], 1.0)

            with tc.tile_pool(name="psum_main", bufs=1, space="PSUM") as psum:
                # ---- masks from xev
                with tc.tile_pool(name="setup", bufs=1) as setup:
                    xev = setup.tile([2, NF], F32, tag="xev_t", bufs=1,
                                     name="xev_t")
                    nc.sync.dma_start(xev[:], d["xev"][:])
                    rows = setup.tile([2, NF], F32, tag="rows", bufs=1,
                                      name="rows")
                    nc.vector.tensor_scalar(rows[:], xev[:], 0.0, None, ALU.is_gt)
                    ones1 = setup.tile([1, 100], F32, tag="ones1", bufs=1,
                                       name="ones1")
                    nc.any.memset(ones1[:], 1.0)
                    # f-batched mask broadcasts (100, NF)
                    for r, dst in [(0, maskS_b), (1, maskC_b)]:
                        for c0 in range(0, NF, 512):
                            pm = psum.tile([100, 512], F32, tag="gg", bufs=2,
                                           name="pmask")
                            nc.tensor.matmul(pm[:], ones1[:],
                                             rows[r:r + 1, c0:c0 + 512],
                                             start=True, stop=True)
                            nc.scalar.copy(dst[:, c0:c0 + 512], pm[:])
                    # per-sample masks (any event over t)
                    sm = setup.tile([2, BC], F32, tag="sm", bufs=1, name="sm")
                    nc.vector.tensor_reduce(
                        sm[:].unsqueeze(2),
                        rows[:].rearrange("p (f k) -> p k f", f=T),
                        mybir.AxisListType.X, ALU.add)
                    smr = setup.tile([2, BC], F32, tag="smr", bufs=1, name="smr")
                    nc.vector.tensor_scalar(smr[:], sm[:], 0.0, None, ALU.is_gt)
                    both = setup.tile([1, BC], F32, tag="both", bufs=1,
                                      name="both")
                    nc.vector.tensor_tensor(both[:], sm[0:1, :], sm[1:2, :],
                                            ALU.add)
                    nc.vector.tensor_scalar(mBoth[:], both[:], 0.0, None,
                                            ALU.is_gt)
                    for r, dst in [(0, mI), (1, mC)]:
                        pm = psum.tile([100, 512], F32, tag="gg", bufs=2,
                                       name="pmask2")
                        nc.tensor.matmul(pm[:, 0:BC], ones1[:], smr[r:r + 1, :],
                                         start=True, stop=True)
                        nc.scalar.copy(dst[:, 0:BC], pm[:, 0:BC])
                        nc.vector.tensor_copy(dst[:, BC:128], dst[:, 0:BC])

                # ---- prologue: fwd-L0 runs for S, C, E (+ E bwd-L0), N=64
                def run_l0_64(wi0_t, wh0_t, d0, rhs_fn, h_dest_fn, ctag):
                    c64 = persist.tile([100, BC], F32, tag=ctag, bufs=1,
                                       name=ctag)
                    h_prev = None
                    for t_ in range(T):
                        parts = [(wi0_t[:, d0 * 400:(d0 + 1) * 400], rhs_fn(t_))]
                        if t_ > 0:
                            parts.append((wh0_t[:, d0 * 400:(d0 + 1) * 400],
                                          h_prev))
                        _emit_cell64(nc, work, psum, parts, 4, c64[:],
                                     h_dest_fn(t_), first=(t_ == 0))
                        h_prev = h_dest_fn(t_)

                run_l0_64(wt["s_wi0"], wt["s_wh0"], 0,
                          lambda t_: xt9[:, t_ * BC:(t_ + 1) * BC],
                          lambda t_: yf1S[0:100, t_ * BC:(t_ + 1) * BC], "c64s")
                run_l0_64(wt["c_wi0"], wt["c_wh0"], 0,
                          lambda t_: xt9[:, t_ * BC:(t_ + 1) * BC],
                          lambda t_: yf1C[0:100, t_ * BC:(t_ + 1) * BC], "c64c")
                run_l0_64(wt["e_wi0"], wt["e_wh0"], 0,
                          lambda t_: xt9[:, t_ * BC:(t_ + 1) * BC],
                          lambda t_: yf1E[0:100, t_ * BC:(t_ + 1) * BC], "c64e")
                run_l0_64(wt["e_wi0"], wt["e_wh0"], 1,
                          lambda t_: xt9[:, (T - 1 - t_) * BC:(T - t_) * BC],
                          lambda t_: bigYE[:, t_ * BC:(t_ + 1) * BC], "c64eb")

                # ---- shift phases (the heavy part)
                s_wts = {k[2:]: v for k, v in wt.items() if k.startswith("s_")}
                c_wts = {k[2:]: v for k, v in wt.items() if k.startswith("c_")}
                with tc.tile_pool(name="shiftS", bufs=1) as sp:
                    _emit_shift_phase(nc, work, psum, sp, xt9, yf1S, s_wts,
                                      maskS_b, s0_bf)
                with tc.tile_pool(name="shiftC", bufs=1) as sp:
                    _emit_shift_phase(nc, work, psum, sp, xt9, yf1C, c_wts,
                                      maskC_b, sc0_bf)

                # ---- encoder fwd L1 + 1-step bwd L1
                cEf1 = P([100, BC], F32, "cEf1")
                hE_prev = None
                for t_ in range(T):
                    hE = work.tile([100, BC], BF16, tag="hEf1", bufs=2,
                                   name="hEf1")
                    parts = [(wt["e_wi1a"][:, 0:400],
                              yf1E[:, t_ * BC:(t_ + 1) * BC]),
                             (wt["e_wi1b"][:, 0:400],
                              bigYE[:, (T - 1 - t_) * BC:(T - t_) * BC])]
                    if t_ > 0:
                        parts.append((wt["e_wh1"][:, 0:400], hE_prev[:]))
                    dest = enc_bf[:, 0:BC] if t_ == T - 1 else hE[:]
                    _emit_cell64(nc, work, psum, parts, 4, cEf1[:], dest,
                                 first=(t_ == 0))
                    hE_prev = hE
                cEb1 = P([100, BC], F32, "cEb1")
                parts = [(wt["e_wi1a"][:, 400:800], yf1E[:, (T - 1) * BC:T * BC]),
                         (wt["e_wi1b"][:, 400:800], bigYE[:, 0:BC])]
                _emit_cell64(nc, work, psum, parts, 4, cEb1[:],
                             enc_bf[:, BC:128], first=True)

            # ---- decoder
            with (
                tc.tile_pool(name="psum_dec", bufs=1, space="PSUM") as psd,
                tc.tile_pool(name="decp", bufs=1) as dp,
            ):
                for name in dec_wnames:
                    dh = d[name]
                    t = dp.tile(list(dh.shape), WDT, tag="w_" + name, bufs=1,
                                name="w_" + name)
                    nc.sync.dma_start(t[:], dh[:])
                    wt[name] = t

                def st_h(name, w):
                    ts = []
                    for k in range(2):
                        t_ = dp.tile([101, w], BF16, tag=f"{name}_{k}", bufs=1,
                                     name=f"{name}_{k}")
                        nc.vector.memset(t_[:], 1.0)
                        nc.vector.memset(t_[0:100, :], 0.0)
                        ts.append(t_)
                    return ts

                def st_c(name, hd):
                    # batch-major cell state: (batch, features)
                    return [dp.tile([BC, hd], F32, tag=f"{name}c_{k}",
                                    bufs=1, name=f"{name}c_{k}")
                            for k in range(2)]

                y_t = st_h("y", 128)
                ssum = dp.tile([101, 128], BF16, tag="ssum", bufs=1, name="ssum")
                nc.vector.memset(ssum[:], 1.0)
                states = {}
                for dec, w, hd in [("d", 64, 100), ("ds", 128, 200),
                                   ("dc", 128, 200)]:
                    for lay in ("0", "1"):
                        for dr in ("f", "b"):
                            states[f"{dec}h{lay}{dr}"] = st_h(f"{dec}h{lay}{dr}", w)
                            states[f"{dec}c{lay}{dr}"] = st_c(f"{dec}c{lay}{dr}", hd)
                nc.vector.tensor_copy(y_t[1][0:100, :], enc_bf[:])

                eye = wt["eye64"]

                def dec_cell(dec, lay, dr, k, xparts, nM, first):
                    """Batch-major cell: lhsT = feature-major inputs (states),
                    rhs = weight chunks (K, 4H), gates psum (64, 4H). h is
                    transposed back to the feature-major state tile via PE."""
                    cur, prev = k % 2, (k + 1) % 2
                    h_prev = states[f"{dec}h{lay}{dr}"][prev]
                    h_out = states[f"{dec}h{lay}{dr}"][cur]
                    c_t = states[f"{dec}c{lay}{dr}"][0]
                    di = 0 if dr == "f" else 1
                    H4 = nM * 100
                    Hd = H4 // 4
                    # chunks: (lhsT_state_ap, rhs_weight_full (K, H4))
                    chunks = [(st, w[:, di * H4:(di + 1) * H4])
                              for (w, st) in xparts(di)]
                    if not first:
                        if dec == "d":
                            chunks.append((h_prev[0:100, :],
                                           wt[f"d_wh{lay}"][:, di * H4:(di + 1) * H4]))
                        else:
                            chunks.append((h_prev[0:100, 0:BC],
                                           wt[f"{dec}_wh{lay}a"][:, di * H4:(di + 1) * H4]))
                            chunks.append((h_prev[0:100, BC:128],
                                           wt[f"{dec}_wh{lay}b"][:, di * H4:(di + 1) * H4]))
                    pg = psd.tile([BC, 1024], F32, tag="bm", bufs=2, name="pgbm")
                    n = len(chunks)
                    for n0 in range(0, H4, 512):
                        n1 = min(n0 + 512, H4)
                        for j, (lh, rr) in enumerate(chunks):
                            nc.tensor.matmul(pg[:, n0:n1], lh, rr[:, n0:n1],
                                             start=(j == 0), stop=(j == n - 1))
                    A = work.tile([BC, 1024], BF16, tag="Abm", bufs=2, name="Abm")
                    nsig = 3 * Hd
                    nc.scalar.activation(A[:, 0:nsig], pg[:, 0:nsig], AF.Sigmoid)
                    nc.scalar.activation(A[:, nsig:H4], pg[:, nsig:H4], AF.Tanh)
                    i_s, f_s, o_s, g_s = (A[:, kk * Hd:(kk + 1) * Hd]
                                          for kk in range(4))
                    cc = c_t[:, 0:Hd]
                    if first:
                        nc.vector.tensor_tensor(cc, i_s, g_s, ALU.mult)
                    else:
                        tig = work.tile([BC, 256], BF16, tag="tigbm", bufs=2,
                                        name="tigbm")
                        nc.vector.tensor_tensor(tig[:, 0:Hd], i_s, g_s, ALU.mult)
                        tfc = work.tile([BC, 256], F32, tag="tfcbm", bufs=2,
                                        name="tfcbm")
                        nc.vector.tensor_tensor(tfc[:, 0:Hd], f_s, cc, ALU.mult)
                        nc.vector.tensor_tensor(cc, tfc[:, 0:Hd], tig[:, 0:Hd],
                                                ALU.add)
                    th = work.tile([BC, 256], BF16, tag="thbm", bufs=2,
                                   name="thbm")
                    nc.scalar.activation(th[:, 0:Hd], cc, AF.Tanh)
                    hbm = work.tile([BC, 256], BF16, tag="hbm", bufs=2,
                                    name="hbm")
                    nc.vector.tensor_tensor(hbm[:, 0:Hd], o_s, th[:, 0:Hd],
                                            ALU.mult)
                    for j in range(Hd // 100):
                        tp = psd.tile([100, BC], BF16, tag="tp", bufs=2,
                                      name="tp")
                        nc.tensor.transpose(tp[:], hbm[:, j * 100:(j + 1) * 100],
                                            eye[:])
                        nc.scalar.copy(h_out[0:100, j * BC:(j + 1) * BC], tp[:])
                    return h_out

                for k in range(FC):
                    cur, prev = k % 2, (k + 1) % 2
                    y_prev = y_t[prev]
                    first = (k == 0)
                    if k == 0:
                        s_a, s_b = s0_bf[:, 0:BC], s0_bf[:, BC:128]
                        sc_a, sc_b = sc0_bf[:, 0:BC], sc0_bf[:, BC:128]
                    else:
                        s_a = states["dsh1b"][prev][0:100, 0:BC]
                        s_b = states["dsh1b"][prev][0:100, BC:128]
                        sc_a = states["dch1b"][prev][0:100, 0:BC]
                        sc_b = states["dch1b"][prev][0:100, BC:128]

                    def d_x0(di, y_prev=y_prev):
                        return [(wt["d_wi0a"], y_prev[0:101, 0:BC]),
                                (wt["d_wi0b"], y_prev[0:100, BC:128])]
                    h0f = dec_cell("d", "0", "f", k, d_x0, 4, first)
                    h0b = dec_cell("d", "0", "b", k, d_x0, 4, first)

                    def d_x1(di, h0f=h0f, h0b=h0b):
                        return [(wt["d_wi1a"], h0f[0:101, :]),
                                (wt["d_wi1b"], h0b[0:100, :])]
                    yd_a = dec_cell("d", "1", "f", k, d_x1, 4, first)
                    yd_b = dec_cell("d", "1", "b", k, d_x1, 4, first)

                    outs = {}
                    for dec, (sa, sb) in [("ds", (s_a, s_b)),
                                          ("dc", (sc_a, sc_b))]:
                        def s_x0(di, dec=dec, sa=sa, sb=sb, y_prev=y_prev):
                            return [
                                (wt[f"{dec}_wi0a"], y_prev[0:101, 0:BC]),
                                (wt[f"{dec}_wi0b"], y_prev[0:100, BC:128]),
                                (wt[f"{dec}_wi0c"], sa),
                                (wt[f"{dec}_wi0d"], sb),
                            ]
                        g0f = dec_cell(dec, "0", "f", k, s_x0, 8, first)
                        g0b = dec_cell(dec, "0", "b", k, s_x0, 8, first)

                        def s_x1(di, dec=dec, g0f=g0f, g0b=g0b):
                            return [
                                (wt[f"{dec}_wi1a"], g0f[0:101, 0:BC]),
                                (wt[f"{dec}_wi1b"], g0f[0:100, BC:128]),
                                (wt[f"{dec}_wi1c"], g0b[0:100, 0:BC]),
                                (wt[f"{dec}_wi1d"], g0b[0:100, BC:128]),
                            ]
                        outs[dec + "f"] = dec_cell(dec, "1", "f", k, s_x1, 8,
                                                   first)
                        outs[dec + "b"] = dec_cell(dec, "1", "b", k, s_x1, 8,
                                                   first)

                    # --- yx = yd - mI*relu(ySf) + mC*relu(ySCf)
                    y_new = y_t[cur]
                    r1 = work.tile([100, 128], BF16, tag="r1", bufs=2, name="r1")
                    nc.vector.tensor_scalar(r1[:], outs["dsf"][0:100, :], 0.0,
                                            None, ALU.max)
                    rm1 = work.tile([100, 128], BF16, tag="rm1", bufs=2,
                                    name="rm1")
                    nc.vector.tensor_tensor(rm1[:], r1[:], mI[:], ALU.mult)
                    nc.vector.tensor_tensor(y_new[0:100, 0:BC], yd_a[0:100, :],
                                            rm1[:, 0:BC], ALU.subtract)
                    nc.vector.tensor_tensor(y_new[0:100, BC:128], yd_b[0:100, :],
                                            rm1[:, BC:128], ALU.subtract)
                    r2 = work.tile([100, 128], BF16, tag="r1", bufs=2, name="r2")
                    nc.vector.tensor_scalar(r2[:], outs["dcf"][0:100, :], 0.0,
                                            None, ALU.max)
                    rm2 = work.tile([100, 128], BF16, tag="rm1", bufs=2,
                                    name="rm2")
                    nc.vector.tensor_tensor(rm2[:], r2[:], mC[:], ALU.mult)
                    nc.vector.tensor_tensor(y_new[0:100, 0:BC],
                                            y_new[0:100, 0:BC],
                                            rm2[:, 0:BC], ALU.add)
                    nc.vector.tensor_tensor(y_new[0:100, BC:128],
                                            y_new[0:100, BC:128],
                                            rm2[:, BC:128], ALU.add)

                    # --- out_f = yx @ lin_w.T + lin_b
                    pl = psd.tile([1, BC], F32, tag="lin", bufs=2, name="pl")
                    nc.tensor.matmul(pl[:], wt["lin"][:, 0:1],
                                     y_new[0:101, 0:BC], start=True, stop=False)
                    nc.tensor.matmul(pl[:], wt["lin"][0:100, 1:2],
                                     y_new[0:100, BC:128], start=False, stop=True)
                    nc.scalar.copy(OUT[:, k * BC:(k + 1) * BC], pl[:])

                    # --- outS = mBoth * ((mI*ySb + mC*ySCb) @ linS_w.T + linS_b)
                    t1 = work.tile([100, 128], BF16, tag="r1", bufs=2, name="t1")
                    nc.vector.tensor_tensor(t1[:], outs["dsb"][0:100, :], mI[:],
                                            ALU.mult)
                    t2 = work.tile([100, 128], BF16, tag="rm1", bufs=2, name="t2")
                    nc.vector.tensor_tensor(t2[:], outs["dcb"][0:100, :], mC[:],
                                            ALU.mult)
                    nc.vector.tensor_tensor(ssum[0:100, :], t1[:], t2[:], ALU.add)
                    pl2 = psd.tile([1, BC], F32, tag="lin", bufs=2, name="pl2")
                    nc.tensor.matmul(pl2[:], wt["lin"][:, 2:3],
                                     ssum[0:101, 0:BC], start=True, stop=False)
                    nc.tensor.matmul(pl2[:], wt["lin"][0:100, 3:4],
                                     ssum[0:100, BC:128], start=False, stop=True)
                    rowS = work.tile([1, BC], F32, tag="rowS", bufs=2,
                                     name="rowS")
                    nc.scalar.copy(rowS[:], pl2[:])
                    nc.vector.tensor_tensor(OUT[:, (FC + k) * BC:(FC + k + 1) * BC],
                                            rowS[:], mBoth[:], ALU.mult)

                nc.sync.dma_start(d["out"][:], OUT[:])
    return nc


_CACHED = None


def _install_ntff_shim():
    """Register a minimal antenv.axon_hooks so trace=True works under axon
    (only used when KERNEL_TRACE=1; the plain run never needs it)."""
    import contextlib
    import ctypes
    import types
    import glob

    hook = None
    cands = glob.glob("/opt/axon/libaxon_pjrt.so") + glob.glob(
        "/root/.axon_site/**/libaxon_pjrt.so", recursive=True)
    for so_path in cands:
        try:
            lib = ctypes.CDLL(so_path)
        except OSError:
            continue
        if not hasattr(lib, "axon_start_nrt_profile"):
            continue
        lib.axon_start_nrt_profile.argtypes = [
            ctypes.POINTER(ctypes.c_int64), ctypes.c_size_t]
        lib.axon_start_nrt_profile.restype = ctypes.c_int64
        lib.axon_stop_nrt_profile.argtypes = [ctypes.c_char_p]
        lib.axon_stop_nrt_profile.restype = ctypes.c_int64

        @contextlib.contextmanager
        def _hook(output_dir, device_ids, lib=lib):
            import jax
            jax.devices()
            if device_ids:
                ids = (ctypes.c_int64 * len(device_ids))(*device_ids)
                rc = lib.axon_start_nrt_profile(ids, len(device_ids))
            else:
                rc = lib.axon_start_nrt_profile(None, 0)
            if rc != 0:
                raise RuntimeError(f"axon_start_nrt_profile rc={rc}")
            try:
                yield
            finally:
                n = lib.axon_stop_nrt_profile(str(output_dir).encode())
                print(f"ntff profile: {n} file(s) -> {output_dir}",
                      file=sys.stderr)
        hook = _hook
        break
    mod = types.ModuleType("antenv.axon_hooks")
    mod.get_axon_ntff_profile_hook = lambda: hook
    mod.set_axon_ntff_profile_hook = lambda h: None
    import antenv
    antenv.axon_hooks = mod
    sys.modules["antenv.axon_hooks"] = mod


def kernel(**inputs):
    global _CACHED
    if _CACHED is None:
        _CACHED = build_kernel()
    nc = _CACHED
    in_maps = _prep_inputs(**inputs)
    trace = os.environ.get("KERNEL_TRACE", "0") == "1"
    if trace:
        _install_ntff_shim()
        r = run_bass_kernel_spmd(nc, in_maps, list(range(NCORES)), trace=True)
        print(f"HW exec time: {r.exec_time_ns} ns")
        res = r.results
    else:
        res = run_bass_kernel_spmd(nc, in_maps, list(range(NCORES))).results
    outer = np.zeros((BP, FC), np.float32)
    outS = np.zeros((BP, FC), np.float32)
    for c in range(NCORES):
        o = res[c]["out"]  # (24, 64)
        outer[c * BC:(c + 1) * BC] = o[0:FC].T
        outS[c * BC:(c + 1) * BC] = o[FC:2 * FC].T
    return outer[:B], outS[:B]
